# revision 10
# baseline (speedup 1.0000x reference)
"""Trainium2 Bass kernel for nn_BuildifyMOE (moe_routing).

Strategy: data-parallel over the batch across 8 NeuronCores (128 tokens/core,
exactly one SBUF partition tile). Token-major layout ([tokens, features]).
All matmuls run as float32r (full PE rate at moving-dim >= 256, ~1e-4 rel
err) with PE-transposed activations as the stationary operand. Biases enter
PSUM via rank-1 (K=1) ones-row matmuls; one-shot bias vectors are streamed
from HBM since a [1, N] SBUF tile reserves N*4 bytes on every partition.
Dense expert compute; expert weights stream from HBM and overlap the
sequential GRU recurrence. Per-step h2^T tiles bounce through DRAM for the
decoder-head phase. The aux_loss batch reduction finishes on host from
per-core partial sums.
"""
import os

import numpy as np

import concourse.bass as bass
import concourse.mybir as mybir
import concourse.tile as tile
from concourse.bass_utils import run_bass_kernel_spmd

P = 128
F_IN = 64
D = 512
HFF = 2048
E = 8
G = 256
T_TYPES = 16
RF = 7
OUT_F = 4 + T_TYPES + 3  # 23
N_CORES = 8

f32 = mybir.dt.float32
f32r = mybir.dt.float32r
bf16 = mybir.dt.bfloat16
AF = mybir.ActivationFunctionType
ALU = mybir.AluOpType
AX = mybir.AxisListType


def _legalize_waits(nc):
    """This container's walrus rejects >1 sync-wait command per instruction.
    Move excess waits onto NoOp carriers inserted before, same engine."""
    n = 0
    for f in nc.m.functions:
        for blk in f.blocks:
            insts = list(blk.instructions)
            out = []
            changed = False
            for inst in insts:
                si = getattr(inst, "sync_info", None)
                waits = list(si.on_wait) if (si is not None and si.on_wait) else []
                if len(waits) > 1:
                    for w in waits[:-1]:
                        nop = mybir.InstNoOp(name=f"I-waitnop-{n}", ins=[], outs=[])
                        n += 1
                        nop.engine = inst.engine
                        nop.sync_info = mybir.SyncInfo(on_wait=[w], on_update=[])
                        out.append(nop)
                    si.on_wait = [waits[-1]]
                    changed = True
                out.append(inst)
            if changed:
                blk.instructions = out
    return n


class _Blob:
    """Accumulates 1-row fp32 constants into one [1, total] array."""

    def __init__(self):
        self.vals = []
        self.offs = {}
        self.total = 0

    def add(self, name, vec):
        vec = np.asarray(vec, np.float32).reshape(-1)
        self.offs[name] = (self.total, vec.size)
        self.vals.append(vec)
        self.total += vec.size

    def array(self):
        return np.concatenate(self.vals)[None, :]


def _build(S, res_offs, res_total, str_offs, str_total):
    nc = bass.Bass()

    dram = {}

    def din(name, shape):
        dram[name] = nc.dram_tensor(name, list(shape), f32, kind="ExternalInput")
        return dram[name]

    din("constraints", [P, F_IN])
    dram["xseq_aug"] = nc.dram_tensor("xseq_aug", [8, S, P], bf16, kind="ExternalInput")          # rows 0-6: x_seq[t].T, row 7: ones
    din("ident", [P, P])
    din("ones_col", [P, 1])
    din("resblob", [1, res_total])      # resident biases (GRU/gate/comb/head)
    din("strblob", [1, str_total])      # streamed one-shot biases
    din("gb", [10, D])                  # gamma/beta rows for the 5 LNs
    din("proj_w", [P, D])               # zero-padded 64->128 on host
    din("attn_w", [D, D])               # wv @ wo folded on host
    din("ff_w1", [D, HFF])
    din("ff_w2", [HFF, D])
    din("gate_w1", [D, G])
    din("gate_w2", [G, E])
    dram["exp_w1"] = nc.dram_tensor("exp_w1", [E, D, HFF], bf16,
                                    kind="ExternalInput")
    dram["exp_w2"] = nc.dram_tensor("exp_w2", [E, HFF, D], bf16,
                                    kind="ExternalInput")
    din("mix_w", [D, D])
    for nm, shp in [("wi0_aug", [8, 3 * D]), ("wh0", [D, 3 * D]),
                    ("wi1", [D, 3 * D]), ("wh1", [D, 3 * D])]:
        dram[nm] = nc.dram_tensor(nm, shp, bf16, kind="ExternalInput")
    dram["comb_w"] = nc.dram_tensor("comb_w", [2 * D, D], bf16,
                                    kind="ExternalInput")
    dram["head_w"] = nc.dram_tensor("head_w", [D, OUT_F + 1], bf16,
                                    kind="ExternalInput")
    din("grub", [5, D])

    rooms_d = nc.dram_tensor("rooms", [P, S, OUT_F], f32, kind="ExternalOutput")
    probs_d = nc.dram_tensor("gate_probs", [P, E], f32, kind="ExternalOutput")
    usage_d = nc.dram_tensor("usage_p", [1, E], f32, kind="ExternalOutput")

    def kmaj(name):
        """DRAM [K, N] -> fp32r AP [128, K/128, N] for SBUF K-chunk layout."""
        ap = dram[name].ap().bitcast(f32r)
        return ap.rearrange("(o p) n -> p o n", p=P)

    def kmaj_bf(name):
        return dram[name].ap().rearrange("(o p) n -> p o n", p=P)

    def gb_bcast(dst, row0, nrows):
        """Broadcast gb rows [row0:row0+nrows] across partitions into dst."""
        src = bass.AP(tensor=dram["gb"], offset=row0 * D,
                      ap=[[0, P], [D, nrows], [1, D]])
        nc.gpsimd.dma_start(dst, src)

    with tile.TileContext(nc) as tc:
        with (
            tc.tile_pool(name="persist", bufs=1) as persist,
            tc.tile_pool(name="work", bufs=6) as work,
            tc.tile_pool(name="bias_sb", bufs=2) as bias_sb,
            tc.tile_pool(name="tp_ps", bufs=2, space="PSUM") as tp_ps,
            tc.tile_pool(name="dram_sc", bufs=1, space="DRAM") as dram_sc,
        ):
            ident = persist.tile([P, P], f32)
            nc.sync.dma_start(ident, dram["ident"].ap())
            rblob = persist.tile([1, res_total], f32r)
            nc.sync.dma_start(rblob, dram["resblob"].ap().bitcast(f32r))
            eps_t = persist.tile([P, 1], f32)
            nc.vector.memset(eps_t, 1e-5)

            def rb(name):
                off, size = res_offs[name]
                return rblob[0:1, off:off + size]

            ones_row = rb("ones")  # [1, 128] of 1.0

            def stream_bias(name):
                """DMA a one-shot bias vector into a rotating [1, *] tile."""
                off, size = str_offs[name]
                t = bias_sb.tile([1, HFF], f32r, name="bstr", tag="bstr")
                nc.sync.dma_start(
                    t[:, :size],
                    dram["strblob"].ap().bitcast(f32r)[0:1, off:off + size])
                return t[0:1, 0:size]

            def mm_group(ps, pairs):
                """Emit one PSUM accumulation group from (lhsT, rhs) pairs."""
                for i, (l, r) in enumerate(pairs):
                    nc.tensor.matmul(ps, l, r, start=(i == 0),
                                     stop=(i == len(pairs) - 1))

            def transpose_into(dstT, src_sb, n_chunks, tag):
                """src_sb [128, n*128] fp32 -> dstT [128, n, 128] fp32r."""
                for c in range(n_chunks):
                    pt = tp_ps.tile([P, P], f32, name="tp", tag="tp")
                    nc.tensor.transpose(pt, src_sb[:, c * P:(c + 1) * P], ident)
                    nc.vector.tensor_copy(dstT[:, c], pt)

            def layernorm(x_sb, out_sb, gbt, gi, bi):
                """out = LN(x) * gbt[:, gi] + gbt[:, bi]."""
                st = work.tile([P, 6], f32, name="lnst", tag="lnst")
                nc.vector.bn_stats(st, x_sb)
                mv = work.tile([P, 2], f32, name="lnmv", tag="lnmv")
                nc.vector.bn_aggr(mv, st)
                rstd = work.tile([P, 1], f32, name="lnrs", tag="lnrs")
                nc.scalar.activation(rstd, mv[:, 1:2], AF.Sqrt, bias=eps_t)
                nc.vector.reciprocal(rstd, rstd)
                nc.vector.tensor_scalar(out_sb, x_sb, mv[:, 0:1], rstd,
                                        op0=ALU.subtract, op1=ALU.mult)
                nc.vector.tensor_mul(out_sb, out_sb, gbt[:, gi])
                nc.vector.tensor_add(out_sb, out_sb, gbt[:, bi])

            # persistent activations
            emb = persist.tile([P, D], f32)
            embT = persist.tile([P, D // P, P], f32r)
            cw = persist.tile([P, E], f32)
            mixed = persist.tile([P, D], f32)
            h2T_dram = dram_sc.tile([S, P, 4, P], bf16)

            # ---------------- Encoder + gating ----------------
            with (
                tc.tile_pool(name="encw", bufs=1) as encw,
                tc.tile_pool(name="encs", bufs=2) as encs,
                tc.tile_pool(name="enc_ps", bufs=2, space="PSUM") as enc_ps,
                tc.tile_pool(name="enc_sb", bufs=1) as enc_sb,
            ):
                gbe = encw.tile([P, 6, D], f32)
                gb_bcast(gbe, 0, 6)
                w_proj = encw.tile([P, D], f32r)
                nc.sync.dma_start(w_proj, dram["proj_w"].ap().bitcast(f32r))
                w_attn = encw.tile([P, 4, D], f32r)
                nc.sync.dma_start(w_attn, kmaj("attn_w"))
                w_g1 = encw.tile([P, 4, G], f32r)
                nc.sync.dma_start(w_g1, kmaj("gate_w1"))
                w_g2 = encw.tile([P, 2, E], f32r)
                nc.sync.dma_start(w_g2, kmaj("gate_w2"))

                cons = enc_sb.tile([P, P], f32)
                nc.vector.memset(cons, 0.0)
                nc.sync.dma_start(cons[:, 0:F_IN], dram["constraints"].ap())
                consT = enc_sb.tile([P, P], f32r)
                pt0 = tp_ps.tile([P, P], f32, name="tp", tag="tp")
                nc.tensor.transpose(pt0, cons, ident)
                nc.vector.tensor_copy(consT, pt0)

                # h = LN0(cons @ proj_w + proj_b)
                ps = enc_ps.tile([P, D], f32, name="eps0", tag="emm")
                mm_group(ps, [(consT, w_proj),
                              (ones_row, stream_bias("proj_b"))])
                h_pre = enc_sb.tile([P, D], f32)
                nc.scalar.activation(h_pre, ps, AF.Copy)
                h_ln = enc_sb.tile([P, D], f32)
                layernorm(h_pre, h_ln, gbe, 0, 1)

                # attn (folded): x1 = h_ln + h_ln @ attn_w + b_attn; h1e = LN1
                hT = enc_sb.tile([P, 4, P], f32r)
                transpose_into(hT, h_ln, 4, "hT")
                ps = enc_ps.tile([P, D], f32, name="eps1", tag="emm")
                mm_group(ps, [(hT[:, k], w_attn[:, k]) for k in range(4)]
                         + [(ones_row, stream_bias("b_attn"))])
                x1 = enc_sb.tile([P, D], f32)
                nc.vector.tensor_tensor(x1, h_ln, ps, ALU.add)
                h1e = enc_sb.tile([P, D], f32)
                layernorm(x1, h1e, gbe, 2, 3)

                # ff: emb = LN2(h1e + gelu(h1e@W1+b1)@W2+b2), W1/W2 streamed
                h1eT = enc_sb.tile([P, 4, P], f32r)
                transpose_into(h1eT, h1e, 4, "h1eT")
                ffh = enc_sb.tile([P, HFF], f32)
                b1 = stream_bias("ff_b1")
                for nb in range(4):
                    wc = encs.tile([P, 4, D], f32r, name="ffw", tag="ffw")
                    nc.sync.dma_start(wc, kmaj("ff_w1")[:, :, nb * D:(nb + 1) * D])
                    psf = enc_ps.tile([P, D], f32, name="epsf", tag="emm")
                    mm_group(psf, [(h1eT[:, k], wc[:, k]) for k in range(4)]
                             + [(ones_row, b1[:, nb * D:(nb + 1) * D])])
                    nc.scalar.activation(ffh[:, nb * D:(nb + 1) * D], psf, AF.Gelu)
                ffhT = enc_sb.tile([P, 16, P], f32r)
                transpose_into(ffhT, ffh, 16, "ffhT")
                ps = enc_ps.tile([P, D], f32, name="eps2", tag="emm")
                for kb in range(4):
                    wc = encs.tile([P, 4, D], f32r, name="ffw2", tag="ffw")
                    nc.sync.dma_start(wc, kmaj("ff_w2")[:, kb * 4:(kb + 1) * 4, :])
                    for k in range(4):
                        nc.tensor.matmul(ps, ffhT[:, kb * 4 + k], wc[:, k],
                                         start=(kb == 0 and k == 0), stop=False)
                nc.tensor.matmul(ps, ones_row, stream_bias("ff_b2"),
                                 start=False, stop=True)
                x2 = enc_sb.tile([P, D], f32)
                nc.vector.tensor_tensor(x2, h1e, ps, ALU.add)
                layernorm(x2, emb, gbe, 4, 5)
                transpose_into(embT, emb, 4, "embT")

                # ---------------- Gating ----------------
                psg = enc_ps.tile([P, G], f32, name="epsg", tag="emm")
                mm_group(psg, [(embT[:, k], w_g1[:, k]) for k in range(4)]
                         + [(ones_row, rb("gate_b1"))])
                g1 = enc_sb.tile([P, G], f32)
                nc.scalar.activation(g1, psg, AF.Gelu)
                g1T = enc_sb.tile([P, 2, P], f32r)
                transpose_into(g1T, g1, 2, "g1T")
                psl = enc_ps.tile([P, E], f32, name="epsl", tag="emm")
                mm_group(psl, [(g1T[:, k], w_g2[:, k]) for k in range(2)]
                         + [(ones_row, rb("gate_b2"))])

                # softmax over E=8 (free axis)
                mx = work.tile([P, 1], f32, name="smx", tag="smx")
                nc.vector.reduce_max(mx, psl, axis=AX.X)
                sh = enc_sb.tile([P, E], f32)
                nc.vector.tensor_scalar(sh, psl, mx, None, op0=ALU.subtract)
                ex = enc_sb.tile([P, E], f32)
                nc.scalar.activation(ex, sh, AF.Exp)
                sm = work.tile([P, 1], f32, name="ssm", tag="ssm")
                nc.vector.reduce_sum(sm, ex, axis=AX.X)
                rs = work.tile([P, 1], f32, name="srs", tag="srs")
                nc.vector.reciprocal(rs, sm)
                probs = enc_sb.tile([P, E], f32)
                nc.vector.tensor_scalar_mul(probs, ex, rs)
                nc.sync.dma_start(probs_d.ap(), probs)

                # usage partial: column sums of probs via ones-column matmul
                ones_col = enc_sb.tile([P, 1], f32r)
                nc.sync.dma_start(ones_col, dram["ones_col"].ap().bitcast(f32r))
                probs_r = enc_sb.tile([P, E], f32r)
                nc.vector.tensor_copy(probs_r, probs)
                psu = enc_ps.tile([1, E], f32, name="epsu", tag="emm")
                nc.tensor.matmul(psu, ones_col, probs_r, start=True, stop=True)
                usage_sb = enc_sb.tile([1, E], f32)
                nc.vector.tensor_copy(usage_sb, psu)
                nc.sync.dma_start(usage_d.ap(), usage_sb)

                # top-2 combine weights cw[b, e] = p_e * in_top2 / (p1 + p2)
                m1 = work.tile([P, 1], f32, name="m1", tag="m1")
                nc.vector.reduce_max(m1, probs, axis=AX.X)
                mask1 = enc_sb.tile([P, E], f32)
                nc.vector.tensor_scalar(mask1, probs, m1, None, op0=ALU.is_equal)
                inv1 = enc_sb.tile([P, E], f32)
                nc.vector.tensor_scalar(inv1, mask1, -1.0, 1.0,
                                        op0=ALU.mult, op1=ALU.add)
                probs2 = enc_sb.tile([P, E], f32)
                nc.vector.tensor_mul(probs2, probs, inv1)
                m2 = work.tile([P, 1], f32, name="m2", tag="m2")
                nc.vector.reduce_max(m2, probs2, axis=AX.X)
                mask2 = enc_sb.tile([P, E], f32)
                nc.vector.tensor_scalar(mask2, probs2, m2, None, op0=ALU.is_equal)
                wmask = enc_sb.tile([P, E], f32)
                nc.vector.tensor_add(wmask, mask1, mask2)
                den = work.tile([P, 1], f32, name="den", tag="den")
                nc.vector.tensor_tensor(den, m1, m2, ALU.add)
                rden = work.tile([P, 1], f32, name="rden", tag="rden")
                nc.vector.reciprocal(rden, den)
                cwt = enc_sb.tile([P, E], f32)
                nc.vector.tensor_mul(cwt, probs, wmask)
                nc.vector.tensor_scalar_mul(cw, cwt, rden)

            # ------- Experts (bf16, interleaved between GRU steps) + GRU
            with (
                tc.tile_pool(name="expw", bufs=2) as expw,
                tc.tile_pool(name="exp_sb", bufs=1) as exp_sb,
                tc.tile_pool(name="exp_ps", bufs=2, space="PSUM") as exp_ps,
                tc.tile_pool(name="gruw", bufs=1) as gruw,
                tc.tile_pool(name="gru_sb", bufs=2) as gru_sb,
                tc.tile_pool(name="gru_gt", bufs=6) as gru_gt,
                tc.tile_pool(name="gru_ps", bufs=4, space="PSUM") as gru_ps,
            ):
                embT_bf = gruw.tile([P, 4, P], bf16)
                nc.vector.tensor_copy(embT_bf, embT)

                def expert(e):
                    h1x = exp_sb.tile([P, HFF], f32, name="h1x", tag="h1x")
                    h1xT = exp_sb.tile([P, 16, P], bf16, name="h1xT",
                                       tag="h1xT")
                    b1e = stream_bias(f"exp_b1_{e}")
                    for nb in range(4):
                        w1c = expw.tile([P, 4, D], bf16, name="w1c", tag="ew")
                        nc.sync.dma_start(
                            w1c,
                            dram["exp_w1"].ap()[e, :, nb * D:(nb + 1) * D]
                            .rearrange("(o p) n -> p o n", p=P))
                        pse = exp_ps.tile([P, D], f32, name="pse", tag="eps")
                        mm_group(pse,
                                 [(embT_bf[:, k], w1c[:, k]) for k in range(4)]
                                 + [(ones_row, b1e[:, nb * D:(nb + 1) * D])])
                        nc.scalar.activation(h1x[:, nb * D:(nb + 1) * D],
                                             pse, AF.Gelu)
                        for c in range(4):
                            cc = nb * 4 + c
                            pt = tp_ps.tile([P, P], f32, name="tp", tag="tp")
                            nc.tensor.transpose(
                                pt, h1x[:, cc * P:(cc + 1) * P], ident)
                            nc.vector.tensor_copy(h1xT[:, cc], pt)
                    pso = exp_ps.tile([P, D], f32, name="pso", tag="eps")
                    for kb in range(4):
                        w2c = expw.tile([P, 4, D], bf16, name="w2c", tag="ew")
                        nc.sync.dma_start(
                            w2c,
                            dram["exp_w2"].ap()[e, kb * D:(kb + 1) * D, :]
                            .rearrange("(o p) n -> p o n", p=P))
                        for k in range(4):
                            nc.tensor.matmul(pso, h1xT[:, kb * 4 + k], w2c[:, k],
                                             start=(kb == 0 and k == 0),
                                             stop=False)
                    nc.tensor.matmul(pso, ones_row, stream_bias(f"exp_b2_{e}"),
                                     start=False, stop=True)
                    if e == 0:
                        nc.vector.tensor_scalar(mixed, pso, cw[:, 0:1], None,
                                                op0=ALU.mult)
                    else:
                        nc.vector.scalar_tensor_tensor(
                            mixed, pso, cw[:, e:e + 1], mixed,
                            op0=ALU.mult, op1=ALU.add)

                # ---- GRU weights + per-gate broadcast biases
                xseqT = gruw.tile([8, S, P], bf16)
                nc.sync.dma_start(xseqT, dram["xseq_aug"].ap())
                wi0a = gruw.tile([8, 3 * D], bf16)
                nc.sync.dma_start(wi0a, dram["wi0_aug"].ap())
                wh0 = gruw.tile([P, 4, 3 * D], bf16)
                nc.sync.dma_start(wh0, kmaj_bf("wh0"))
                wi1 = gruw.tile([P, 4, 3 * D], bf16)
                nc.sync.dma_start(wi1, kmaj_bf("wi1"))
                wh1 = gruw.tile([P, 4, 3 * D], bf16)
                nc.sync.dma_start(wh1, kmaj_bf("wh1"))
                # grub rows: 0 c2-r bias, 1 c2-z bias, 2 c2-inn, 3 c1-hn, 4 c2-hn
                grub = gruw.tile([P, 5, D], f32)
                nc.gpsimd.dma_start(
                    grub, bass.AP(tensor=dram["grub"], offset=0,
                                  ap=[[0, P], [D, 5], [1, D]]))

                def gru_cell(ci, gates_mms, h_sb, rz_bias, i_bias, h_bias):
                    """gates_mms: gate -> [(lhsT, rhs)] matmul pairs.
                    *_bias: broadcast [128, D] slices added on DVE (or None)."""
                    pss = {}
                    for gname in ("r", "z", "i", "h"):
                        ps = gru_ps.tile([P, D], f32, name=f"ps{gname}{ci}",
                                         tag="gps")
                        mm_group(ps, gates_mms[gname])
                        pss[gname] = ps
                    if rz_bias is not None:
                        rpre = gru_gt.tile([P, D], f32, name=f"rp{ci}", tag="gt")
                        nc.vector.tensor_tensor(rpre, pss["r"], rz_bias[0],
                                                ALU.add)
                        zpre = gru_gt.tile([P, D], f32, name=f"zp{ci}", tag="gt")
                        nc.vector.tensor_tensor(zpre, pss["z"], rz_bias[1],
                                                ALU.add)
                    else:
                        rpre, zpre = pss["r"], pss["z"]
                    r_sb = gru_gt.tile([P, D], f32, name=f"r{ci}", tag="gt")
                    nc.scalar.activation(r_sb, rpre, AF.Sigmoid)
                    z_sb = gru_gt.tile([P, D], f32, name=f"z{ci}", tag="gt")
                    nc.scalar.activation(z_sb, zpre, AF.Sigmoid)
                    hb = gru_gt.tile([P, D], f32, name=f"hb{ci}", tag="gt")
                    nc.vector.tensor_tensor(hb, pss["h"], h_bias, ALU.add)
                    t1 = gru_gt.tile([P, D], f32, name=f"t1{ci}", tag="gt")
                    nc.vector.tensor_mul(t1, r_sb, hb)
                    nc.vector.tensor_tensor(t1, t1, pss["i"], ALU.add)
                    if i_bias is not None:
                        nc.vector.tensor_tensor(t1, t1, i_bias, ALU.add)
                    n_sb = gru_gt.tile([P, D], f32, name=f"n{ci}", tag="gt")
                    nc.scalar.activation(n_sb, t1, AF.Tanh)
                    # h_new = n + z * (h - n)
                    dd = gru_gt.tile([P, D], f32, name=f"d{ci}", tag="gt")
                    nc.vector.tensor_sub(dd, h_sb, n_sb)
                    nc.vector.tensor_mul(dd, z_sb, dd)
                    hnew = gru_sb.tile([P, D], f32, name=f"hs{ci}",
                                       tag=f"hs{ci}")
                    nc.vector.tensor_add(hnew, n_sb, dd)
                    return hnew

                h1_sb = gru_sb.tile([P, D], f32, name="hs1", tag="hs1")
                nc.vector.memset(h1_sb, 0.0)
                h2_sb = gru_sb.tile([P, D], f32, name="hs2", tag="hs2")
                nc.vector.memset(h2_sb, 0.0)
                h1T = None
                h2T_prev = None

                for t in range(S):
                    # interleave one expert every other step so its matmuls
                    # fill PE gaps in the recurrence and its weight DMA
                    # streams behind the compute
                    if t % 2 == 0 and t // 2 < E:
                        expert(t // 2)

                    xs = xseqT[:, t, :]
                    g1m = {
                        "r": [(xs, wi0a[:, 0:D])],
                        "z": [(xs, wi0a[:, D:2 * D])],
                        "i": [(xs, wi0a[:, 2 * D:3 * D])],
                        "h": [],
                    }
                    if h1T is not None:
                        for k in range(4):
                            g1m["r"].append((h1T[:, k], wh0[:, k, 0:D]))
                            g1m["z"].append((h1T[:, k], wh0[:, k, D:2 * D]))
                            g1m["h"].append((h1T[:, k], wh0[:, k, 2 * D:3 * D]))
                    else:
                        g1m["h"] = [(ones_row, rb("zeroD"))]
                    h1_sb = gru_cell(1, g1m, h1_sb, None, None, grub[:, 3])
                    h1T = gru_sb.tile([P, 4, P], bf16, name="h1T", tag="h1T")
                    transpose_into(h1T, h1_sb, 4, "h1T")

                    g2m = {
                        "r": [(h1T[:, k], wi1[:, k, 0:D]) for k in range(4)],
                        "z": [(h1T[:, k], wi1[:, k, D:2 * D]) for k in range(4)],
                        "i": [(h1T[:, k], wi1[:, k, 2 * D:3 * D])
                              for k in range(4)],
                        "h": [],
                    }
                    if h2T_prev is not None:
                        for k in range(4):
                            g2m["r"].append((h2T_prev[:, k], wh1[:, k, 0:D]))
                            g2m["z"].append((h2T_prev[:, k],
                                             wh1[:, k, D:2 * D]))
                            g2m["h"].append((h2T_prev[:, k],
                                             wh1[:, k, 2 * D:3 * D]))
                    else:
                        g2m["h"] = [(ones_row, rb("zeroD"))]
                    h2_sb = gru_cell(2, g2m, h2_sb,
                                     (grub[:, 0], grub[:, 1]), grub[:, 2],
                                     grub[:, 4])
                    h2T = gru_sb.tile([P, 4, P], bf16, name="h2T", tag="h2T")
                    transpose_into(h2T, h2_sb, 4, "h2T")
                    nc.sync.dma_start(h2T_dram[t], h2T)
                    h2T_prev = h2T

            # ---------------- context = gelu(LN(mixed@mix_w + mix_b)) + emb
            ctx = persist.tile([P, D], f32)
            ctxT = persist.tile([P, D // P, P], bf16)
            with (
                tc.tile_pool(name="mixw", bufs=1) as mixw,
                tc.tile_pool(name="mix_ps", bufs=2, space="PSUM") as mix_ps,
                tc.tile_pool(name="mix_sb", bufs=1) as mix_sb,
            ):
                gbm = mixw.tile([P, 2, D], f32)
                gb_bcast(gbm, 6, 2)
                w_mix = mixw.tile([P, 4, D], f32r)
                nc.sync.dma_start(w_mix, kmaj("mix_w"))
                mixT = mix_sb.tile([P, 4, P], f32r)
                transpose_into(mixT, mixed, 4, "mixT")
                ps = mix_ps.tile([P, D], f32, name="mps", tag="mmm")
                mm_group(ps, [(mixT[:, k], w_mix[:, k]) for k in range(4)]
                         + [(ones_row, stream_bias("mix_b"))])
                mx_sb = mix_sb.tile([P, D], f32)
                nc.scalar.activation(mx_sb, ps, AF.Copy)
                mxn = mix_sb.tile([P, D], f32)
                layernorm(mx_sb, mxn, gbm, 0, 1)
                mxg = mix_sb.tile([P, D], f32)
                nc.scalar.activation(mxg, mxn, AF.Gelu)
                nc.vector.tensor_tensor(ctx, mxg, emb, ALU.add)
                transpose_into(ctxT, ctx, 4, "ctxT")

            # ---------------- decoder heads: comb + output heads per step
            with (
                tc.tile_pool(name="combw", bufs=1) as combw,
                tc.tile_pool(name="cmb_ps", bufs=4, space="PSUM") as cmb_ps,
                tc.tile_pool(name="cmb_sb", bufs=3) as cmb_sb,
            ):
                gbc = combw.tile([P, 2, D], f32)
                gb_bcast(gbc, 8, 2)
                w_comb = combw.tile([P, 8, D], bf16)
                nc.sync.dma_start(w_comb, kmaj_bf("comb_w"))
                w_head = combw.tile([P, 4, OUT_F + 1], bf16)
                nc.sync.dma_start(w_head, kmaj_bf("head_w"))
                rooms_sb = combw.tile([P, S, OUT_F], f32)

                for t in range(S):
                    h2Tt = cmb_sb.tile([P, 4, P], bf16, name="h2Tt", tag="h2Tt")
                    nc.sync.dma_start(h2Tt, h2T_dram[t])
                    ps = cmb_ps.tile([P, D], f32, name="cps", tag="cmm")
                    mm_group(ps,
                             [(ctxT[:, k], w_comb[:, k]) for k in range(4)]
                             + [(h2Tt[:, k], w_comb[:, 4 + k])
                                for k in range(4)]
                             + [(ones_row, rb("comb_b"))])
                    cm = cmb_sb.tile([P, D], f32, name="cm", tag="cm")
                    nc.scalar.activation(cm, ps, AF.Copy)
                    cn = cmb_sb.tile([P, D], f32, name="cn", tag="cn")
                    layernorm(cm, cn, gbc, 0, 1)
                    hid = cmb_sb.tile([P, D], f32, name="hid", tag="hid")
                    nc.scalar.activation(hid, cn, AF.Gelu)
                    hidT = cmb_sb.tile([P, 4, P], bf16, name="hidT", tag="hidT")
                    transpose_into(hidT, hid, 4, "hidT")
                    psh = cmb_ps.tile([P, OUT_F + 1], f32, name="hps", tag="cmm")
                    mm_group(psh,
                             [(hidT[:, k], w_head[:, k]) for k in range(4)]
                             + [(ones_row, rb("head_b"))])
                    nc.scalar.activation(rooms_sb[:, t, 0:4], psh[:, 0:4],
                                         AF.Sigmoid)
                    nc.scalar.activation(rooms_sb[:, t, 4:20], psh[:, 4:20],
                                         AF.Copy)
                    nc.scalar.activation(rooms_sb[:, t, 20:22], psh[:, 20:22],
                                         AF.Sigmoid)
                    nc.scalar.activation(rooms_sb[:, t, 22:23], psh[:, 22:23],
                                         AF.Copy)
                nc.sync.dma_start(rooms_d.ap(), rooms_sb)

    _legalize_waits(nc)
    return nc


_BUILD_CACHE = {}
LAST_EXEC_NS = None


def _make_blobs(p):
    res = _Blob()
    res.add("ones", np.ones(P))
    res.add("zeroD", np.zeros(D))
    res.add("gate_b1", p["gate_b1"])
    res.add("gate_b2", p["gate_b2"])
    res.add("bh0_n", p["gru_bh0"][2 * D:])
    res.add("bi1bh1_rz", (p["gru_bi1"] + p["gru_bh1"])[:2 * D])
    res.add("bi1_n", p["gru_bi1"][2 * D:])
    res.add("bh1_n", p["gru_bh1"][2 * D:])
    res.add("comb_b", p["comb_b"])
    res.add("head_b", np.concatenate(
        [p["coord_b"], p["type_b"], p["zone_b"], p["ext_b"], p["stop_b"],
         np.zeros(1, np.float32)]))

    st = _Blob()
    st.add("proj_b", p["proj_b"])
    st.add("b_attn", p["attn_bv"] @ p["attn_wo"] + p["attn_bo"])
    st.add("ff_b1", p["ff_b1"])
    st.add("ff_b2", p["ff_b2"])
    st.add("mix_b", p["mix_b"])
    for e in range(E):
        st.add(f"exp_b1_{e}", p["exp_b1"][e])
        st.add(f"exp_b2_{e}", p["exp_b2"][e])
    return res, st


def kernel(constraints, teacher_rooms, params, num_rooms):
    constraints = np.ascontiguousarray(np.asarray(constraints, np.float32))
    teacher_rooms = np.ascontiguousarray(np.asarray(teacher_rooms, np.float32))
    p = {k: np.ascontiguousarray(np.asarray(v, np.float32))
         for k, v in params.items()}
    S = int(num_rooms)
    B = constraints.shape[0]
    Bl = B // N_CORES
    import ml_dtypes
    bfl = ml_dtypes.bfloat16

    res, st = _make_blobs(p)

    gb = np.ascontiguousarray(np.stack(
        [p["ln0_g"], p["ln0_b"], p["ln1_g"], p["ln1_b"],
         p["ln2_g"], p["ln2_b"], p["mix_ln_g"], p["mix_ln_b"],
         p["comb_ln_g"], p["comb_ln_b"]]))

    proj_w_pad = np.zeros((P, D), np.float32)
    proj_w_pad[:F_IN] = p["proj_w"]
    attn_w = np.ascontiguousarray(p["attn_wv"] @ p["attn_wo"])

    wi0_aug = np.zeros((8, 3 * D), np.float32)
    wi0_aug[:RF] = p["gru_wi0"]
    wi0_aug[RF, :2 * D] = (p["gru_bi0"] + p["gru_bh0"])[:2 * D]
    wi0_aug[RF, 2 * D:] = p["gru_bi0"][2 * D:]

    head_w = np.ascontiguousarray(np.concatenate(
        [p["coord_w"], p["type_w"], p["zone_w"], p["ext_w"], p["stop_w"],
         np.zeros((D, 1), np.float32)], 1))

    rzsum = (p["gru_bi1"] + p["gru_bh1"])
    grub = np.ascontiguousarray(np.stack(
        [rzsum[0:D], rzsum[D:2 * D], p["gru_bi1"][2 * D:],
         p["gru_bh0"][2 * D:], p["gru_bh1"][2 * D:]]))

    key = S
    if key not in _BUILD_CACHE:
        _BUILD_CACHE[key] = _build(S, res.offs, res.total, st.offs, st.total)
    nc = _BUILD_CACHE[key]

    shared = {
        "ident": np.eye(P, dtype=np.float32),
        "ones_col": np.ones((P, 1), np.float32),
        "resblob": np.ascontiguousarray(res.array()),
        "strblob": np.ascontiguousarray(st.array()),
        "gb": gb,
        "proj_w": proj_w_pad,
        "attn_w": attn_w,
        "ff_w1": p["ff_w1"], "ff_w2": p["ff_w2"],
        "gate_w1": p["gate_w1"], "gate_w2": p["gate_w2"],
        "exp_w1": np.ascontiguousarray(p["exp_w1"].astype(bfl)),
        "exp_w2": np.ascontiguousarray(p["exp_w2"].astype(bfl)),
        "grub": grub,
        "mix_w": p["mix_w"],
        "wi0_aug": np.ascontiguousarray(wi0_aug.astype(bfl)),
        "wh0": np.ascontiguousarray(p["gru_wh0"].astype(bfl)),
        "wi1": np.ascontiguousarray(p["gru_wi1"].astype(bfl)),
        "wh1": np.ascontiguousarray(p["gru_wh1"].astype(bfl)),
        "comb_w": np.ascontiguousarray(p["comb_w"].astype(bfl)),
        "head_w": np.ascontiguousarray(head_w.astype(bfl)),
    }

    in_maps = []
    for i in range(N_CORES):
        sl = slice(i * Bl, (i + 1) * Bl)
        xseq_aug = np.zeros((8, S, P), np.float32)
        tr = teacher_rooms[sl]
        for t in range(1, S):
            xseq_aug[:RF, t, :] = tr[:, t - 1, :].T
        xseq_aug[RF, :, :] = 1.0
        m = dict(shared)
        m["constraints"] = np.ascontiguousarray(constraints[sl])
        m["xseq_aug"] = xseq_aug.astype(bfl)
        in_maps.append(m)

    global LAST_EXEC_NS
    profile = bool(os.environ.get("KERNEL_PROFILE"))
    res_hw = run_bass_kernel_spmd(nc, in_maps, core_ids=list(range(N_CORES)),
                                  trace=profile)
    LAST_EXEC_NS = res_hw.exec_time_ns

    rooms = np.concatenate([r["rooms"] for r in res_hw.results], 0)
    gate_probs = np.concatenate([r["gate_probs"] for r in res_hw.results], 0)
    usage = sum(r["usage_p"][0] for r in res_hw.results) / B
    aux = np.float32(np.mean((usage - 1.0 / E) ** 2) * E)
    return rooms, gate_probs, aux


# revision 11
# speedup vs baseline: 1.2721x; 1.2721x over previous
"""Trainium2 Bass kernel for nn_BuildifyMOE (moe_routing).

Strategy: data-parallel over the batch across 8 NeuronCores (128 tokens/core,
exactly one SBUF partition tile). Token-major layout ([tokens, features]).
All matmuls run as float32r (full PE rate at moving-dim >= 256, ~1e-4 rel
err) with PE-transposed activations as the stationary operand. Biases enter
PSUM via rank-1 (K=1) ones-row matmuls; one-shot bias vectors are streamed
from HBM since a [1, N] SBUF tile reserves N*4 bytes on every partition.
Dense expert compute; expert weights stream from HBM and overlap the
sequential GRU recurrence. Per-step h2^T tiles bounce through DRAM for the
decoder-head phase. The aux_loss batch reduction finishes on host from
per-core partial sums.
"""
import os

import numpy as np

import concourse.bass as bass
import concourse.mybir as mybir
import concourse.tile as tile
from concourse.bass_utils import run_bass_kernel_spmd

P = 128
F_IN = 64
D = 512
HFF = 2048
E = 8
G = 256
T_TYPES = 16
RF = 7
OUT_F = 4 + T_TYPES + 3  # 23
N_CORES = 8

f32 = mybir.dt.float32
f32r = mybir.dt.float32r
bf16 = mybir.dt.bfloat16
AF = mybir.ActivationFunctionType
ALU = mybir.AluOpType
AX = mybir.AxisListType


def _legalize_waits(nc):
    """This container's walrus rejects >1 sync-wait command per instruction.
    Move excess waits onto NoOp carriers inserted before, same engine."""
    n = 0
    for f in nc.m.functions:
        for blk in f.blocks:
            insts = list(blk.instructions)
            out = []
            changed = False
            for inst in insts:
                si = getattr(inst, "sync_info", None)
                waits = list(si.on_wait) if (si is not None and si.on_wait) else []
                if len(waits) > 1:
                    for w in waits[:-1]:
                        nop = mybir.InstNoOp(name=f"I-waitnop-{n}", ins=[], outs=[])
                        n += 1
                        nop.engine = inst.engine
                        nop.sync_info = mybir.SyncInfo(on_wait=[w], on_update=[])
                        out.append(nop)
                    si.on_wait = [waits[-1]]
                    changed = True
                out.append(inst)
            if changed:
                blk.instructions = out
    return n


class _Blob:
    """Accumulates 1-row fp32 constants into one [1, total] array."""

    def __init__(self):
        self.vals = []
        self.offs = {}
        self.total = 0

    def add(self, name, vec):
        vec = np.asarray(vec, np.float32).reshape(-1)
        self.offs[name] = (self.total, vec.size)
        self.vals.append(vec)
        self.total += vec.size

    def array(self):
        return np.concatenate(self.vals)[None, :]


def _build(S, res_offs, res_total, str_offs, str_total):
    nc = bass.Bass()

    dram = {}

    def din(name, shape):
        dram[name] = nc.dram_tensor(name, list(shape), f32, kind="ExternalInput")
        return dram[name]

    din("constraints", [P, F_IN])
    dram["xseq_aug"] = nc.dram_tensor("xseq_aug", [8, S, P], bf16, kind="ExternalInput")          # rows 0-6: x_seq[t].T, row 7: ones
    din("ident", [P, P])
    din("ones_col", [P, 1])
    din("resblob", [1, res_total])      # resident biases (GRU/gate/comb/head)
    din("strblob", [1, str_total])      # streamed one-shot biases
    din("gb", [10, D])                  # gamma/beta rows for the 5 LNs
    din("proj_w", [P, D])               # zero-padded 64->128 on host
    din("attn_w", [D, D])               # wv @ wo folded on host
    din("ff_w1", [D, HFF])
    din("ff_w2", [HFF, D])
    din("gate_w1", [D, G])
    din("gate_w2", [G, E])
    dram["exp_w1"] = nc.dram_tensor("exp_w1", [E, D, HFF], bf16,
                                    kind="ExternalInput")
    dram["exp_w2"] = nc.dram_tensor("exp_w2", [E, HFF, D], bf16,
                                    kind="ExternalInput")
    din("mix_w", [D, D])
    for nm, shp in [("wi0_aug", [8, 3 * D]), ("wh0", [D, 3 * D]),
                    ("wi1", [D, 3 * D]), ("wh1", [D, 3 * D])]:
        dram[nm] = nc.dram_tensor(nm, shp, bf16, kind="ExternalInput")
    dram["comb_w"] = nc.dram_tensor("comb_w", [2 * D, D], bf16,
                                    kind="ExternalInput")
    dram["head_w"] = nc.dram_tensor("head_w", [D, OUT_F + 1], bf16,
                                    kind="ExternalInput")
    din("grub", [5, D])

    rooms_d = nc.dram_tensor("rooms", [P, S, OUT_F], f32, kind="ExternalOutput")
    probs_d = nc.dram_tensor("gate_probs", [P, E], f32, kind="ExternalOutput")
    usage_d = nc.dram_tensor("usage_p", [1, E], f32, kind="ExternalOutput")

    def kmaj(name):
        """DRAM [K, N] -> fp32r AP [128, K/128, N] for SBUF K-chunk layout."""
        ap = dram[name].ap().bitcast(f32r)
        return ap.rearrange("(o p) n -> p o n", p=P)

    def kmaj_bf(name):
        return dram[name].ap().rearrange("(o p) n -> p o n", p=P)

    def gb_bcast(dst, row0, nrows):
        """Broadcast gb rows [row0:row0+nrows] across partitions into dst."""
        src = bass.AP(tensor=dram["gb"], offset=row0 * D,
                      ap=[[0, P], [D, nrows], [1, D]])
        nc.gpsimd.dma_start(dst, src)

    with tile.TileContext(nc) as tc:
        with (
            tc.tile_pool(name="persist", bufs=1) as persist,
            tc.tile_pool(name="work", bufs=6) as work,
            tc.tile_pool(name="bias_sb", bufs=2) as bias_sb,
            tc.tile_pool(name="tp_ps", bufs=2, space="PSUM") as tp_ps,
            tc.tile_pool(name="dram_sc", bufs=1, space="DRAM") as dram_sc,
        ):
            ident = persist.tile([P, P], f32)
            nc.sync.dma_start(ident, dram["ident"].ap())
            rblob = persist.tile([1, res_total], f32r)
            nc.sync.dma_start(rblob, dram["resblob"].ap().bitcast(f32r))
            eps_t = persist.tile([P, 1], f32)
            nc.vector.memset(eps_t, 1e-5)

            def rb(name):
                off, size = res_offs[name]
                return rblob[0:1, off:off + size]

            ones_row = rb("ones")  # [1, 128] of 1.0

            def stream_bias(name):
                """DMA a one-shot bias vector into a rotating [1, *] tile."""
                off, size = str_offs[name]
                t = bias_sb.tile([1, HFF], f32r, name="bstr", tag="bstr")
                nc.sync.dma_start(
                    t[:, :size],
                    dram["strblob"].ap().bitcast(f32r)[0:1, off:off + size])
                return t[0:1, 0:size]

            def mm_group(ps, pairs):
                """Emit one PSUM accumulation group from (lhsT, rhs) pairs."""
                for i, (l, r) in enumerate(pairs):
                    nc.tensor.matmul(ps, l, r, start=(i == 0),
                                     stop=(i == len(pairs) - 1))

            def transpose_into(dstT, src_sb, n_chunks, tag):
                """src_sb [128, n*128] fp32 -> dstT [128, n, 128] fp32r."""
                for c in range(n_chunks):
                    pt = tp_ps.tile([P, P], f32, name="tp", tag="tp")
                    nc.tensor.transpose(pt, src_sb[:, c * P:(c + 1) * P], ident)
                    nc.vector.tensor_copy(dstT[:, c], pt)

            def layernorm(x_sb, out_sb, gbt, gi, bi):
                """out = LN(x) * gbt[:, gi] + gbt[:, bi]."""
                st = work.tile([P, 6], f32, name="lnst", tag="lnst")
                nc.vector.bn_stats(st, x_sb)
                mv = work.tile([P, 2], f32, name="lnmv", tag="lnmv")
                nc.vector.bn_aggr(mv, st)
                rstd = work.tile([P, 1], f32, name="lnrs", tag="lnrs")
                nc.scalar.activation(rstd, mv[:, 1:2], AF.Sqrt, bias=eps_t)
                nc.vector.reciprocal(rstd, rstd)
                nc.vector.tensor_scalar(out_sb, x_sb, mv[:, 0:1], rstd,
                                        op0=ALU.subtract, op1=ALU.mult)
                nc.vector.tensor_mul(out_sb, out_sb, gbt[:, gi])
                nc.vector.tensor_add(out_sb, out_sb, gbt[:, bi])

            # persistent activations
            emb = persist.tile([P, D], f32)
            embT = persist.tile([P, D // P, P], f32r)
            cw = persist.tile([P, E], f32)
            mixed = persist.tile([P, D], f32)
            h2T_dram = dram_sc.tile([S, P, 4, P], bf16)

            # ---------------- Encoder + gating ----------------
            with (
                tc.tile_pool(name="encw", bufs=1) as encw,
                tc.tile_pool(name="encs", bufs=2) as encs,
                tc.tile_pool(name="enc_ps", bufs=2, space="PSUM") as enc_ps,
                tc.tile_pool(name="enc_sb", bufs=1) as enc_sb,
            ):
                gbe = encw.tile([P, 6, D], f32)
                gb_bcast(gbe, 0, 6)
                w_proj = encw.tile([P, D], f32r)
                nc.sync.dma_start(w_proj, dram["proj_w"].ap().bitcast(f32r))
                w_attn = encw.tile([P, 4, D], f32r)
                nc.sync.dma_start(w_attn, kmaj("attn_w"))
                w_g1 = encw.tile([P, 4, G], f32r)
                nc.sync.dma_start(w_g1, kmaj("gate_w1"))
                w_g2 = encw.tile([P, 2, E], f32r)
                nc.sync.dma_start(w_g2, kmaj("gate_w2"))

                cons = enc_sb.tile([P, P], f32)
                nc.vector.memset(cons, 0.0)
                nc.sync.dma_start(cons[:, 0:F_IN], dram["constraints"].ap())
                consT = enc_sb.tile([P, P], f32r)
                pt0 = tp_ps.tile([P, P], f32, name="tp", tag="tp")
                nc.tensor.transpose(pt0, cons, ident)
                nc.vector.tensor_copy(consT, pt0)

                # h = LN0(cons @ proj_w + proj_b)
                ps = enc_ps.tile([P, D], f32, name="eps0", tag="emm")
                mm_group(ps, [(consT, w_proj),
                              (ones_row, stream_bias("proj_b"))])
                h_pre = enc_sb.tile([P, D], f32)
                nc.scalar.activation(h_pre, ps, AF.Copy)
                h_ln = enc_sb.tile([P, D], f32)
                layernorm(h_pre, h_ln, gbe, 0, 1)

                # attn (folded): x1 = h_ln + h_ln @ attn_w + b_attn; h1e = LN1
                hT = enc_sb.tile([P, 4, P], f32r)
                transpose_into(hT, h_ln, 4, "hT")
                ps = enc_ps.tile([P, D], f32, name="eps1", tag="emm")
                mm_group(ps, [(hT[:, k], w_attn[:, k]) for k in range(4)]
                         + [(ones_row, stream_bias("b_attn"))])
                x1 = enc_sb.tile([P, D], f32)
                nc.vector.tensor_tensor(x1, h_ln, ps, ALU.add)
                h1e = enc_sb.tile([P, D], f32)
                layernorm(x1, h1e, gbe, 2, 3)

                # ff: emb = LN2(h1e + gelu(h1e@W1+b1)@W2+b2), W1/W2 streamed
                h1eT = enc_sb.tile([P, 4, P], f32r)
                transpose_into(h1eT, h1e, 4, "h1eT")
                ffh = enc_sb.tile([P, HFF], f32)
                b1 = stream_bias("ff_b1")
                for nb in range(4):
                    wc = encs.tile([P, 4, D], f32r, name="ffw", tag="ffw")
                    nc.sync.dma_start(wc, kmaj("ff_w1")[:, :, nb * D:(nb + 1) * D])
                    psf = enc_ps.tile([P, D], f32, name="epsf", tag="emm")
                    mm_group(psf, [(h1eT[:, k], wc[:, k]) for k in range(4)]
                             + [(ones_row, b1[:, nb * D:(nb + 1) * D])])
                    nc.scalar.activation(ffh[:, nb * D:(nb + 1) * D], psf, AF.Gelu)
                ffhT = enc_sb.tile([P, 16, P], f32r)
                transpose_into(ffhT, ffh, 16, "ffhT")
                ps = enc_ps.tile([P, D], f32, name="eps2", tag="emm")
                for kb in range(4):
                    wc = encs.tile([P, 4, D], f32r, name="ffw2", tag="ffw")
                    nc.sync.dma_start(wc, kmaj("ff_w2")[:, kb * 4:(kb + 1) * 4, :])
                    for k in range(4):
                        nc.tensor.matmul(ps, ffhT[:, kb * 4 + k], wc[:, k],
                                         start=(kb == 0 and k == 0), stop=False)
                nc.tensor.matmul(ps, ones_row, stream_bias("ff_b2"),
                                 start=False, stop=True)
                x2 = enc_sb.tile([P, D], f32)
                nc.vector.tensor_tensor(x2, h1e, ps, ALU.add)
                layernorm(x2, emb, gbe, 4, 5)
                transpose_into(embT, emb, 4, "embT")

                # ---------------- Gating ----------------
                psg = enc_ps.tile([P, G], f32, name="epsg", tag="emm")
                mm_group(psg, [(embT[:, k], w_g1[:, k]) for k in range(4)]
                         + [(ones_row, rb("gate_b1"))])
                g1 = enc_sb.tile([P, G], f32)
                nc.scalar.activation(g1, psg, AF.Gelu)
                g1T = enc_sb.tile([P, 2, P], f32r)
                transpose_into(g1T, g1, 2, "g1T")
                psl = enc_ps.tile([P, E], f32, name="epsl", tag="emm")
                mm_group(psl, [(g1T[:, k], w_g2[:, k]) for k in range(2)]
                         + [(ones_row, rb("gate_b2"))])

                # softmax over E=8 (free axis)
                mx = work.tile([P, 1], f32, name="smx", tag="smx")
                nc.vector.reduce_max(mx, psl, axis=AX.X)
                sh = enc_sb.tile([P, E], f32)
                nc.vector.tensor_scalar(sh, psl, mx, None, op0=ALU.subtract)
                ex = enc_sb.tile([P, E], f32)
                nc.scalar.activation(ex, sh, AF.Exp)
                sm = work.tile([P, 1], f32, name="ssm", tag="ssm")
                nc.vector.reduce_sum(sm, ex, axis=AX.X)
                rs = work.tile([P, 1], f32, name="srs", tag="srs")
                nc.vector.reciprocal(rs, sm)
                probs = enc_sb.tile([P, E], f32)
                nc.vector.tensor_scalar_mul(probs, ex, rs)
                nc.sync.dma_start(probs_d.ap(), probs)

                # usage partial: column sums of probs via ones-column matmul
                ones_col = enc_sb.tile([P, 1], f32r)
                nc.sync.dma_start(ones_col, dram["ones_col"].ap().bitcast(f32r))
                probs_r = enc_sb.tile([P, E], f32r)
                nc.vector.tensor_copy(probs_r, probs)
                psu = enc_ps.tile([1, E], f32, name="epsu", tag="emm")
                nc.tensor.matmul(psu, ones_col, probs_r, start=True, stop=True)
                usage_sb = enc_sb.tile([1, E], f32)
                nc.vector.tensor_copy(usage_sb, psu)
                nc.sync.dma_start(usage_d.ap(), usage_sb)

                # top-2 combine weights cw[b, e] = p_e * in_top2 / (p1 + p2)
                m1 = work.tile([P, 1], f32, name="m1", tag="m1")
                nc.vector.reduce_max(m1, probs, axis=AX.X)
                mask1 = enc_sb.tile([P, E], f32)
                nc.vector.tensor_scalar(mask1, probs, m1, None, op0=ALU.is_equal)
                inv1 = enc_sb.tile([P, E], f32)
                nc.vector.tensor_scalar(inv1, mask1, -1.0, 1.0,
                                        op0=ALU.mult, op1=ALU.add)
                probs2 = enc_sb.tile([P, E], f32)
                nc.vector.tensor_mul(probs2, probs, inv1)
                m2 = work.tile([P, 1], f32, name="m2", tag="m2")
                nc.vector.reduce_max(m2, probs2, axis=AX.X)
                mask2 = enc_sb.tile([P, E], f32)
                nc.vector.tensor_scalar(mask2, probs2, m2, None, op0=ALU.is_equal)
                wmask = enc_sb.tile([P, E], f32)
                nc.vector.tensor_add(wmask, mask1, mask2)
                den = work.tile([P, 1], f32, name="den", tag="den")
                nc.vector.tensor_tensor(den, m1, m2, ALU.add)
                rden = work.tile([P, 1], f32, name="rden", tag="rden")
                nc.vector.reciprocal(rden, den)
                cwt = enc_sb.tile([P, E], f32)
                nc.vector.tensor_mul(cwt, probs, wmask)
                nc.vector.tensor_scalar_mul(cw, cwt, rden)

            # ------- Experts (bf16, interleaved between GRU steps) + GRU
            with (
                tc.tile_pool(name="expw", bufs=2) as expw,
                tc.tile_pool(name="exp_sb", bufs=1) as exp_sb,
                tc.tile_pool(name="exp_ps", bufs=2, space="PSUM") as exp_ps,
                tc.tile_pool(name="gruw", bufs=1) as gruw,
                tc.tile_pool(name="gru_sb", bufs=2) as gru_sb,
                tc.tile_pool(name="gru_gt", bufs=6) as gru_gt,
                tc.tile_pool(name="gru_ps", bufs=4, space="PSUM") as gru_ps,
            ):
                embT_bf = gruw.tile([P, 4, P], bf16)
                nc.vector.tensor_copy(embT_bf, embT)

                def expert(e):
                    h1x = exp_sb.tile([P, HFF], f32, name="h1x", tag="h1x")
                    h1xT = exp_sb.tile([P, 16, P], bf16, name="h1xT",
                                       tag="h1xT")
                    b1e = stream_bias(f"exp_b1_{e}")
                    for nb in range(4):
                        w1c = expw.tile([P, 4, D], bf16, name="w1c", tag="ew")
                        nc.sync.dma_start(
                            w1c,
                            dram["exp_w1"].ap()[e, :, nb * D:(nb + 1) * D]
                            .rearrange("(o p) n -> p o n", p=P))
                        pse = exp_ps.tile([P, D], f32, name="pse", tag="eps")
                        mm_group(pse,
                                 [(embT_bf[:, k], w1c[:, k]) for k in range(4)]
                                 + [(ones_row, b1e[:, nb * D:(nb + 1) * D])])
                        nc.scalar.activation(h1x[:, nb * D:(nb + 1) * D],
                                             pse, AF.Gelu)
                        for c in range(4):
                            cc = nb * 4 + c
                            pt = tp_ps.tile([P, P], f32, name="tp", tag="tp")
                            nc.tensor.transpose(
                                pt, h1x[:, cc * P:(cc + 1) * P], ident)
                            nc.vector.tensor_copy(h1xT[:, cc], pt)
                    pso = exp_ps.tile([P, D], f32, name="pso", tag="eps")
                    for kb in range(4):
                        w2c = expw.tile([P, 4, D], bf16, name="w2c", tag="ew")
                        nc.sync.dma_start(
                            w2c,
                            dram["exp_w2"].ap()[e, kb * D:(kb + 1) * D, :]
                            .rearrange("(o p) n -> p o n", p=P))
                        for k in range(4):
                            nc.tensor.matmul(pso, h1xT[:, kb * 4 + k], w2c[:, k],
                                             start=(kb == 0 and k == 0),
                                             stop=False)
                    nc.tensor.matmul(pso, ones_row, stream_bias(f"exp_b2_{e}"),
                                     start=False, stop=True)
                    if e == 0:
                        nc.vector.tensor_scalar(mixed, pso, cw[:, 0:1], None,
                                                op0=ALU.mult)
                    else:
                        nc.vector.scalar_tensor_tensor(
                            mixed, pso, cw[:, e:e + 1], mixed,
                            op0=ALU.mult, op1=ALU.add)

                # ---- GRU weights + per-gate broadcast biases
                xseqT = gruw.tile([8, S, P], bf16)
                nc.sync.dma_start(xseqT, dram["xseq_aug"].ap())
                wi0a = gruw.tile([8, 3 * D], bf16)
                nc.sync.dma_start(wi0a, dram["wi0_aug"].ap())
                wh0 = gruw.tile([P, 4, 3 * D], bf16)
                nc.sync.dma_start(wh0, kmaj_bf("wh0"))
                wi1 = gruw.tile([P, 4, 3 * D], bf16)
                nc.sync.dma_start(wi1, kmaj_bf("wi1"))
                wh1 = gruw.tile([P, 4, 3 * D], bf16)
                nc.sync.dma_start(wh1, kmaj_bf("wh1"))
                # grub rows: 0 c2-r bias, 1 c2-z bias, 2 c2-inn, 3 c1-hn, 4 c2-hn
                grub = gruw.tile([P, 5, D], f32)
                nc.gpsimd.dma_start(
                    grub, bass.AP(tensor=dram["grub"], offset=0,
                                  ap=[[0, P], [D, 5], [1, D]]))

                def gru_cell(ci, gates_mms, h_sb, h1p_sb, rz_bias, i_bias,
                             h_bias):
                    """sigma-only GRU cell (tanh(v) = 2*sigmoid(2v) - 1 keeps
                    the sigmoid LUT resident on ACT). h1p_sb = h_sb + 1.
                    Returns (h_new, h_new + 1)."""
                    pss = {}
                    for gname in ("r", "z", "i", "h"):
                        ps = gru_ps.tile([P, D], f32, name=f"ps{gname}{ci}",
                                         tag="gps")
                        mm_group(ps, gates_mms[gname])
                        pss[gname] = ps
                    if rz_bias is not None:
                        rpre = gru_gt.tile([P, D], f32, name=f"rp{ci}", tag="gt")
                        nc.vector.tensor_tensor(rpre, pss["r"], rz_bias[0],
                                                ALU.add)
                        zpre = gru_gt.tile([P, D], f32, name=f"zp{ci}", tag="gt")
                        nc.vector.tensor_tensor(zpre, pss["z"], rz_bias[1],
                                                ALU.add)
                    else:
                        rpre, zpre = pss["r"], pss["z"]
                    r_sb = gru_gt.tile([P, D], f32, name=f"r{ci}", tag="gt")
                    nc.scalar.activation(r_sb, rpre, AF.Sigmoid)
                    z_sb = gru_gt.tile([P, D], f32, name=f"z{ci}", tag="gt")
                    nc.scalar.activation(z_sb, zpre, AF.Sigmoid)
                    hb = gru_gt.tile([P, D], f32, name=f"hb{ci}", tag="gt")
                    nc.vector.tensor_tensor(hb, pss["h"], h_bias, ALU.add)
                    t1 = gru_gt.tile([P, D], f32, name=f"t1{ci}", tag="gt")
                    nc.vector.tensor_mul(t1, r_sb, hb)
                    nc.vector.tensor_tensor(t1, t1, pss["i"], ALU.add)
                    if i_bias is not None:
                        nc.vector.tensor_tensor(t1, t1, i_bias, ALU.add)
                    # n = tanh(t1) = 2*s - 1 with s = sigmoid(2*t1)
                    s_sb = gru_gt.tile([P, D], f32, name=f"s{ci}", tag="gt")
                    nc.scalar.activation(s_sb, t1, AF.Sigmoid, scale=2.0)
                    # h_new = n + z*(h - n); h - n = (h + 1) - 2s
                    dd = gru_gt.tile([P, D], f32, name=f"d{ci}", tag="gt")
                    nc.vector.scalar_tensor_tensor(dd, s_sb, -2.0, h1p_sb,
                                                   op0=ALU.mult, op1=ALU.add)
                    nc.vector.tensor_mul(dd, z_sb, dd)
                    hnew = gru_sb.tile([P, D], f32, name=f"hs{ci}",
                                       tag=f"hs{ci}")
                    nc.vector.scalar_tensor_tensor(hnew, s_sb, 2.0, dd,
                                                   op0=ALU.mult, op1=ALU.add)
                    nc.vector.tensor_scalar_add(hnew, hnew, -1.0)
                    h1p_new = gru_sb.tile([P, D], f32, name=f"hp{ci}",
                                          tag=f"hp{ci}")
                    nc.vector.tensor_scalar_add(h1p_new, hnew, 1.0)
                    return hnew, h1p_new

                h1_sb = gru_sb.tile([P, D], f32, name="hs1", tag="hs1")
                nc.vector.memset(h1_sb, 0.0)
                h2_sb = gru_sb.tile([P, D], f32, name="hs2", tag="hs2")
                nc.vector.memset(h2_sb, 0.0)
                h1p_sb = gru_sb.tile([P, D], f32, name="hp1", tag="hp1")
                nc.vector.memset(h1p_sb, 1.0)
                h2p_sb = gru_sb.tile([P, D], f32, name="hp2", tag="hp2")
                nc.vector.memset(h2p_sb, 1.0)
                h1T = None
                h2T_prev = None

                for t in range(S):
                    # interleave one expert every other step so its matmuls
                    # fill PE gaps in the recurrence and its weight DMA
                    # streams behind the compute
                    if t % 2 == 0 and t // 2 < E:
                        expert(t // 2)

                    xs = xseqT[:, t, :]
                    g1m = {
                        "r": [(xs, wi0a[:, 0:D])],
                        "z": [(xs, wi0a[:, D:2 * D])],
                        "i": [(xs, wi0a[:, 2 * D:3 * D])],
                        "h": [],
                    }
                    if h1T is not None:
                        for k in range(4):
                            g1m["r"].append((h1T[:, k], wh0[:, k, 0:D]))
                            g1m["z"].append((h1T[:, k], wh0[:, k, D:2 * D]))
                            g1m["h"].append((h1T[:, k], wh0[:, k, 2 * D:3 * D]))
                    else:
                        g1m["h"] = [(ones_row, rb("zeroD"))]
                    h1_sb, h1p_sb = gru_cell(1, g1m, h1_sb, h1p_sb, None, None, grub[:, 3])
                    h1T = gru_sb.tile([P, 4, P], bf16, name="h1T", tag="h1T")
                    transpose_into(h1T, h1_sb, 4, "h1T")

                    g2m = {
                        "r": [(h1T[:, k], wi1[:, k, 0:D]) for k in range(4)],
                        "z": [(h1T[:, k], wi1[:, k, D:2 * D]) for k in range(4)],
                        "i": [(h1T[:, k], wi1[:, k, 2 * D:3 * D])
                              for k in range(4)],
                        "h": [],
                    }
                    if h2T_prev is not None:
                        for k in range(4):
                            g2m["r"].append((h2T_prev[:, k], wh1[:, k, 0:D]))
                            g2m["z"].append((h2T_prev[:, k],
                                             wh1[:, k, D:2 * D]))
                            g2m["h"].append((h2T_prev[:, k],
                                             wh1[:, k, 2 * D:3 * D]))
                    else:
                        g2m["h"] = [(ones_row, rb("zeroD"))]
                    h2_sb, h2p_sb = gru_cell(2, g2m, h2_sb, h2p_sb,
                                             (grub[:, 0], grub[:, 1]),
                                             grub[:, 2], grub[:, 4])
                    h2T = gru_sb.tile([P, 4, P], bf16, name="h2T", tag="h2T")
                    transpose_into(h2T, h2_sb, 4, "h2T")
                    nc.sync.dma_start(h2T_dram[t], h2T)
                    h2T_prev = h2T

            # ---------------- context = gelu(LN(mixed@mix_w + mix_b)) + emb
            ctx = persist.tile([P, D], f32)
            ctxT = persist.tile([P, D // P, P], bf16)
            with (
                tc.tile_pool(name="mixw", bufs=1) as mixw,
                tc.tile_pool(name="mix_ps", bufs=2, space="PSUM") as mix_ps,
                tc.tile_pool(name="mix_sb", bufs=1) as mix_sb,
            ):
                gbm = mixw.tile([P, 2, D], f32)
                gb_bcast(gbm, 6, 2)
                w_mix = mixw.tile([P, 4, D], f32r)
                nc.sync.dma_start(w_mix, kmaj("mix_w"))
                mixT = mix_sb.tile([P, 4, P], f32r)
                transpose_into(mixT, mixed, 4, "mixT")
                ps = mix_ps.tile([P, D], f32, name="mps", tag="mmm")
                mm_group(ps, [(mixT[:, k], w_mix[:, k]) for k in range(4)]
                         + [(ones_row, stream_bias("mix_b"))])
                mx_sb = mix_sb.tile([P, D], f32)
                nc.scalar.activation(mx_sb, ps, AF.Copy)
                mxn = mix_sb.tile([P, D], f32)
                layernorm(mx_sb, mxn, gbm, 0, 1)
                mxg = mix_sb.tile([P, D], f32)
                nc.scalar.activation(mxg, mxn, AF.Gelu)
                nc.vector.tensor_tensor(ctx, mxg, emb, ALU.add)
                transpose_into(ctxT, ctx, 4, "ctxT")

            # ---------------- decoder heads, staged so each ACT function
            # runs as one homogeneous batch (one LUT load per function)
            with (
                tc.tile_pool(name="combw", bufs=1) as combw,
                tc.tile_pool(name="cmb_ps", bufs=4, space="PSUM") as cmb_ps,
                tc.tile_pool(name="cmb_sb", bufs=3) as cmb_sb,
            ):
                gbc = combw.tile([P, 2, D], f32)
                gb_bcast(gbc, 8, 2)
                w_comb = combw.tile([P, 8, D], bf16)
                nc.sync.dma_start(w_comb, kmaj_bf("comb_w"))
                w_head = combw.tile([P, 4, OUT_F + 1], bf16)
                nc.sync.dma_start(w_head, kmaj_bf("head_w"))
                cm_all = combw.tile([P, S, D], f32)
                rooms_raw = combw.tile([P, S, OUT_F + 1], f32)

                h2Tts = []
                for t in range(S):
                    h2Tt = cmb_sb.tile([P, 4, P], bf16, name="h2Tt",
                                       tag=f"h2Tt{t % 4}")
                    nc.sync.dma_start(h2Tt, h2T_dram[t])
                    h2Tts.append(h2Tt)
                    ps = cmb_ps.tile([P, D], f32, name="cps", tag="cmm")
                    mm_group(ps,
                             [(ctxT[:, k], w_comb[:, k]) for k in range(4)]
                             + [(h2Tt[:, k], w_comb[:, 4 + k])
                                for k in range(4)]
                             + [(ones_row, rb("comb_b"))])
                    nc.vector.tensor_copy(cm_all[:, t], ps)
                for t in range(S):
                    layernorm(cm_all[:, t], cm_all[:, t], gbc, 0, 1)
                for t in range(S):
                    nc.scalar.activation(cm_all[:, t], cm_all[:, t], AF.Gelu)
                for t in range(S):
                    hidT = cmb_sb.tile([P, 4, P], bf16, name="hidT",
                                       tag="hidT")
                    transpose_into(hidT, cm_all[:, t], 4, "hidT")
                    psh = cmb_ps.tile([P, OUT_F + 1], f32, name="hps",
                                      tag="cmm")
                    mm_group(psh,
                             [(hidT[:, k], w_head[:, k]) for k in range(4)]
                             + [(ones_row, rb("head_b"))])
                    nc.vector.tensor_copy(rooms_raw[:, t], psh)
                nc.scalar.activation(rooms_raw[:, :, 0:4], rooms_raw[:, :, 0:4],
                                     AF.Sigmoid)
                nc.scalar.activation(rooms_raw[:, :, 20:22],
                                     rooms_raw[:, :, 20:22], AF.Sigmoid)
                nc.sync.dma_start(rooms_d.ap(), rooms_raw[:, :, 0:OUT_F])

    _legalize_waits(nc)
    return nc


_BUILD_CACHE = {}
LAST_EXEC_NS = None


def _make_blobs(p):
    res = _Blob()
    res.add("ones", np.ones(P))
    res.add("zeroD", np.zeros(D))
    res.add("gate_b1", p["gate_b1"])
    res.add("gate_b2", p["gate_b2"])
    res.add("bh0_n", p["gru_bh0"][2 * D:])
    res.add("bi1bh1_rz", (p["gru_bi1"] + p["gru_bh1"])[:2 * D])
    res.add("bi1_n", p["gru_bi1"][2 * D:])
    res.add("bh1_n", p["gru_bh1"][2 * D:])
    res.add("comb_b", p["comb_b"])
    res.add("head_b", np.concatenate(
        [p["coord_b"], p["type_b"], p["zone_b"], p["ext_b"], p["stop_b"],
         np.zeros(1, np.float32)]))

    st = _Blob()
    st.add("proj_b", p["proj_b"])
    st.add("b_attn", p["attn_bv"] @ p["attn_wo"] + p["attn_bo"])
    st.add("ff_b1", p["ff_b1"])
    st.add("ff_b2", p["ff_b2"])
    st.add("mix_b", p["mix_b"])
    for e in range(E):
        st.add(f"exp_b1_{e}", p["exp_b1"][e])
        st.add(f"exp_b2_{e}", p["exp_b2"][e])
    return res, st


def kernel(constraints, teacher_rooms, params, num_rooms):
    constraints = np.ascontiguousarray(np.asarray(constraints, np.float32))
    teacher_rooms = np.ascontiguousarray(np.asarray(teacher_rooms, np.float32))
    p = {k: np.ascontiguousarray(np.asarray(v, np.float32))
         for k, v in params.items()}
    S = int(num_rooms)
    B = constraints.shape[0]
    Bl = B // N_CORES
    import ml_dtypes
    bfl = ml_dtypes.bfloat16

    res, st = _make_blobs(p)

    gb = np.ascontiguousarray(np.stack(
        [p["ln0_g"], p["ln0_b"], p["ln1_g"], p["ln1_b"],
         p["ln2_g"], p["ln2_b"], p["mix_ln_g"], p["mix_ln_b"],
         p["comb_ln_g"], p["comb_ln_b"]]))

    proj_w_pad = np.zeros((P, D), np.float32)
    proj_w_pad[:F_IN] = p["proj_w"]
    attn_w = np.ascontiguousarray(p["attn_wv"] @ p["attn_wo"])

    wi0_aug = np.zeros((8, 3 * D), np.float32)
    wi0_aug[:RF] = p["gru_wi0"]
    wi0_aug[RF, :2 * D] = (p["gru_bi0"] + p["gru_bh0"])[:2 * D]
    wi0_aug[RF, 2 * D:] = p["gru_bi0"][2 * D:]

    head_w = np.ascontiguousarray(np.concatenate(
        [p["coord_w"], p["type_w"], p["zone_w"], p["ext_w"], p["stop_w"],
         np.zeros((D, 1), np.float32)], 1))

    rzsum = (p["gru_bi1"] + p["gru_bh1"])
    grub = np.ascontiguousarray(np.stack(
        [rzsum[0:D], rzsum[D:2 * D], p["gru_bi1"][2 * D:],
         p["gru_bh0"][2 * D:], p["gru_bh1"][2 * D:]]))

    key = S
    if key not in _BUILD_CACHE:
        _BUILD_CACHE[key] = _build(S, res.offs, res.total, st.offs, st.total)
    nc = _BUILD_CACHE[key]

    shared = {
        "ident": np.eye(P, dtype=np.float32),
        "ones_col": np.ones((P, 1), np.float32),
        "resblob": np.ascontiguousarray(res.array()),
        "strblob": np.ascontiguousarray(st.array()),
        "gb": gb,
        "proj_w": proj_w_pad,
        "attn_w": attn_w,
        "ff_w1": p["ff_w1"], "ff_w2": p["ff_w2"],
        "gate_w1": p["gate_w1"], "gate_w2": p["gate_w2"],
        "exp_w1": np.ascontiguousarray(p["exp_w1"].astype(bfl)),
        "exp_w2": np.ascontiguousarray(p["exp_w2"].astype(bfl)),
        "grub": grub,
        "mix_w": p["mix_w"],
        "wi0_aug": np.ascontiguousarray(wi0_aug.astype(bfl)),
        "wh0": np.ascontiguousarray(p["gru_wh0"].astype(bfl)),
        "wi1": np.ascontiguousarray(p["gru_wi1"].astype(bfl)),
        "wh1": np.ascontiguousarray(p["gru_wh1"].astype(bfl)),
        "comb_w": np.ascontiguousarray(p["comb_w"].astype(bfl)),
        "head_w": np.ascontiguousarray(head_w.astype(bfl)),
    }

    in_maps = []
    for i in range(N_CORES):
        sl = slice(i * Bl, (i + 1) * Bl)
        xseq_aug = np.zeros((8, S, P), np.float32)
        tr = teacher_rooms[sl]
        for t in range(1, S):
            xseq_aug[:RF, t, :] = tr[:, t - 1, :].T
        xseq_aug[RF, :, :] = 1.0
        m = dict(shared)
        m["constraints"] = np.ascontiguousarray(constraints[sl])
        m["xseq_aug"] = xseq_aug.astype(bfl)
        in_maps.append(m)

    global LAST_EXEC_NS
    profile = bool(os.environ.get("KERNEL_PROFILE"))
    res_hw = run_bass_kernel_spmd(nc, in_maps, core_ids=list(range(N_CORES)),
                                  trace=profile)
    LAST_EXEC_NS = res_hw.exec_time_ns

    rooms = np.concatenate([r["rooms"] for r in res_hw.results], 0)
    gate_probs = np.concatenate([r["gate_probs"] for r in res_hw.results], 0)
    usage = sum(r["usage_p"][0] for r in res_hw.results) / B
    aux = np.float32(np.mean((usage - 1.0 / E) ** 2) * E)
    return rooms, gate_probs, aux


# revision 12
# speedup vs baseline: 1.3667x; 1.0743x over previous
"""Trainium2 Bass kernel for nn_BuildifyMOE (moe_routing).

Strategy: data-parallel over the batch across 8 NeuronCores (128 tokens/core,
exactly one SBUF partition tile). Token-major layout ([tokens, features]).
All matmuls run as float32r (full PE rate at moving-dim >= 256, ~1e-4 rel
err) with PE-transposed activations as the stationary operand. Biases enter
PSUM via rank-1 (K=1) ones-row matmuls; one-shot bias vectors are streamed
from HBM since a [1, N] SBUF tile reserves N*4 bytes on every partition.
Dense expert compute; expert weights stream from HBM and overlap the
sequential GRU recurrence. Per-step h2^T tiles bounce through DRAM for the
decoder-head phase. The aux_loss batch reduction finishes on host from
per-core partial sums.
"""
import os

import numpy as np

import concourse.bass as bass
import concourse.mybir as mybir
import concourse.tile as tile
from concourse.bass_utils import run_bass_kernel_spmd

P = 128
F_IN = 64
D = 512
HFF = 2048
E = 8
G = 256
T_TYPES = 16
RF = 7
OUT_F = 4 + T_TYPES + 3  # 23
N_CORES = 8

f32 = mybir.dt.float32
f32r = mybir.dt.float32r
bf16 = mybir.dt.bfloat16
AF = mybir.ActivationFunctionType
ALU = mybir.AluOpType
AX = mybir.AxisListType


def _legalize_waits(nc):
    """This container's walrus rejects >1 sync-wait command per instruction.
    Move excess waits onto NoOp carriers inserted before, same engine."""
    n = 0
    for f in nc.m.functions:
        for blk in f.blocks:
            insts = list(blk.instructions)
            out = []
            changed = False
            for inst in insts:
                si = getattr(inst, "sync_info", None)
                waits = list(si.on_wait) if (si is not None and si.on_wait) else []
                if len(waits) > 1:
                    for w in waits[:-1]:
                        nop = mybir.InstNoOp(name=f"I-waitnop-{n}", ins=[], outs=[])
                        n += 1
                        nop.engine = inst.engine
                        nop.sync_info = mybir.SyncInfo(on_wait=[w], on_update=[])
                        out.append(nop)
                    si.on_wait = [waits[-1]]
                    changed = True
                out.append(inst)
            if changed:
                blk.instructions = out
    return n


class _Blob:
    """Accumulates 1-row fp32 constants into one [1, total] array."""

    def __init__(self):
        self.vals = []
        self.offs = {}
        self.total = 0

    def add(self, name, vec):
        vec = np.asarray(vec, np.float32).reshape(-1)
        self.offs[name] = (self.total, vec.size)
        self.vals.append(vec)
        self.total += vec.size

    def array(self):
        return np.concatenate(self.vals)[None, :]


def _build(S, res_offs, res_total, str_offs, str_total):
    nc = bass.Bass()

    dram = {}

    def din(name, shape):
        dram[name] = nc.dram_tensor(name, list(shape), f32, kind="ExternalInput")
        return dram[name]

    din("constraints", [P, F_IN])
    dram["xseq_aug"] = nc.dram_tensor("xseq_aug", [8, S, P], bf16, kind="ExternalInput")          # rows 0-6: x_seq[t].T, row 7: ones
    din("ident", [P, P])
    din("ones_col", [P, 1])
    din("resblob", [1, res_total])      # resident biases (GRU/gate/comb/head)
    din("strblob", [1, str_total])      # streamed one-shot biases
    din("gb", [10, D])                  # gamma/beta rows for the 5 LNs
    din("proj_w", [P, D])               # zero-padded 64->128 on host
    din("attn_w", [D, D])               # wv @ wo folded on host
    din("ff_w1", [D, HFF])
    din("ff_w2", [HFF, D])
    din("gate_w1", [D, G])
    din("gate_w2", [G, E])
    dram["exp_w1"] = nc.dram_tensor("exp_w1", [E, D, HFF], bf16,
                                    kind="ExternalInput")
    dram["exp_w2"] = nc.dram_tensor("exp_w2", [E, HFF, D], bf16,
                                    kind="ExternalInput")
    din("mix_w", [D, D])
    for nm, shp in [("wi0_aug", [8, 3 * D]), ("wh0", [D, 3 * D]),
                    ("wi1", [D, 3 * D]), ("wh1", [D, 3 * D])]:
        dram[nm] = nc.dram_tensor(nm, shp, bf16, kind="ExternalInput")
    dram["comb_w"] = nc.dram_tensor("comb_w", [2 * D, D], bf16,
                                    kind="ExternalInput")
    dram["head_w"] = nc.dram_tensor("head_w", [D, OUT_F + 1], bf16,
                                    kind="ExternalInput")
    din("grub", [5, D])

    rooms_d = nc.dram_tensor("rooms", [P, S, OUT_F], f32, kind="ExternalOutput")
    probs_d = nc.dram_tensor("gate_probs", [P, E], f32, kind="ExternalOutput")
    usage_d = nc.dram_tensor("usage_p", [1, E], f32, kind="ExternalOutput")

    def kmaj(name):
        """DRAM [K, N] -> fp32r AP [128, K/128, N] for SBUF K-chunk layout."""
        ap = dram[name].ap().bitcast(f32r)
        return ap.rearrange("(o p) n -> p o n", p=P)

    def kmaj_bf(name):
        return dram[name].ap().rearrange("(o p) n -> p o n", p=P)

    def gb_bcast(dst, row0, nrows):
        """Broadcast gb rows [row0:row0+nrows] across partitions into dst."""
        src = bass.AP(tensor=dram["gb"], offset=row0 * D,
                      ap=[[0, P], [D, nrows], [1, D]])
        nc.gpsimd.dma_start(dst, src)

    with tile.TileContext(nc) as tc:
        with (
            tc.tile_pool(name="persist", bufs=1) as persist,
            tc.tile_pool(name="work", bufs=6) as work,
            tc.tile_pool(name="bias_sb", bufs=2) as bias_sb,
            tc.tile_pool(name="tp_ps", bufs=2, space="PSUM") as tp_ps,
            tc.tile_pool(name="dram_sc", bufs=1, space="DRAM") as dram_sc,
        ):
            ident = persist.tile([P, P], f32)
            nc.sync.dma_start(ident, dram["ident"].ap())
            rblob = persist.tile([1, res_total], f32r)
            nc.sync.dma_start(rblob, dram["resblob"].ap().bitcast(f32r))
            eps_t = persist.tile([P, 1], f32)
            nc.vector.memset(eps_t, 1e-5)

            def rb(name):
                off, size = res_offs[name]
                return rblob[0:1, off:off + size]

            ones_row = rb("ones")  # [1, 128] of 1.0

            def stream_bias(name):
                """DMA a one-shot bias vector into a rotating [1, *] tile."""
                off, size = str_offs[name]
                t = bias_sb.tile([1, HFF], f32r, name="bstr", tag="bstr")
                nc.sync.dma_start(
                    t[:, :size],
                    dram["strblob"].ap().bitcast(f32r)[0:1, off:off + size])
                return t[0:1, 0:size]

            def mm_group(ps, pairs):
                """Emit one PSUM accumulation group from (lhsT, rhs) pairs."""
                for i, (l, r) in enumerate(pairs):
                    nc.tensor.matmul(ps, l, r, start=(i == 0),
                                     stop=(i == len(pairs) - 1))

            def transpose_into(dstT, src_sb, n_chunks, tag):
                """src_sb [128, n*128] fp32 -> dstT [128, n, 128] fp32r."""
                for c in range(n_chunks):
                    pt = tp_ps.tile([P, P], f32, name="tp", tag="tp")
                    nc.tensor.transpose(pt, src_sb[:, c * P:(c + 1) * P], ident)
                    nc.vector.tensor_copy(dstT[:, c], pt)

            def layernorm(x_sb, out_sb, gbt, gi, bi):
                """out = LN(x) * gbt[:, gi] + gbt[:, bi]."""
                st = work.tile([P, 6], f32, name="lnst", tag="lnst")
                nc.vector.bn_stats(st, x_sb)
                mv = work.tile([P, 2], f32, name="lnmv", tag="lnmv")
                nc.vector.bn_aggr(mv, st)
                rstd = work.tile([P, 1], f32, name="lnrs", tag="lnrs")
                nc.scalar.activation(rstd, mv[:, 1:2], AF.Sqrt, bias=eps_t)
                nc.vector.reciprocal(rstd, rstd)
                nc.vector.tensor_scalar(out_sb, x_sb, mv[:, 0:1], rstd,
                                        op0=ALU.subtract, op1=ALU.mult)
                nc.vector.tensor_mul(out_sb, out_sb, gbt[:, gi])
                nc.vector.tensor_add(out_sb, out_sb, gbt[:, bi])

            # persistent activations
            emb = persist.tile([P, D], f32)
            embT = persist.tile([P, D // P, P], f32r)
            cw = persist.tile([P, E], f32)
            mixed = persist.tile([P, D], f32)
            h2T_dram = dram_sc.tile([S, P, 4, P], bf16)

            # ---------------- Encoder + gating ----------------
            with (
                tc.tile_pool(name="encw", bufs=1) as encw,
                tc.tile_pool(name="encs", bufs=2) as encs,
                tc.tile_pool(name="enc_ps", bufs=2, space="PSUM") as enc_ps,
                tc.tile_pool(name="enc_sb", bufs=1) as enc_sb,
            ):
                gbe = encw.tile([P, 6, D], f32)
                gb_bcast(gbe, 0, 6)
                w_proj = encw.tile([P, D], f32r)
                nc.sync.dma_start(w_proj, dram["proj_w"].ap().bitcast(f32r))
                w_attn = encw.tile([P, 4, D], f32r)
                nc.sync.dma_start(w_attn, kmaj("attn_w"))
                w_g1 = encw.tile([P, 4, G], f32r)
                nc.sync.dma_start(w_g1, kmaj("gate_w1"))
                w_g2 = encw.tile([P, 2, E], f32r)
                nc.sync.dma_start(w_g2, kmaj("gate_w2"))

                cons = enc_sb.tile([P, P], f32)
                nc.vector.memset(cons, 0.0)
                nc.sync.dma_start(cons[:, 0:F_IN], dram["constraints"].ap())
                consT = enc_sb.tile([P, P], f32r)
                pt0 = tp_ps.tile([P, P], f32, name="tp", tag="tp")
                nc.tensor.transpose(pt0, cons, ident)
                nc.vector.tensor_copy(consT, pt0)

                # h = LN0(cons @ proj_w + proj_b)
                ps = enc_ps.tile([P, D], f32, name="eps0", tag="emm")
                mm_group(ps, [(consT, w_proj),
                              (ones_row, stream_bias("proj_b"))])
                h_pre = enc_sb.tile([P, D], f32)
                nc.scalar.activation(h_pre, ps, AF.Copy)
                h_ln = enc_sb.tile([P, D], f32)
                layernorm(h_pre, h_ln, gbe, 0, 1)

                # attn (folded): x1 = h_ln + h_ln @ attn_w + b_attn; h1e = LN1
                hT = enc_sb.tile([P, 4, P], f32r)
                transpose_into(hT, h_ln, 4, "hT")
                ps = enc_ps.tile([P, D], f32, name="eps1", tag="emm")
                mm_group(ps, [(hT[:, k], w_attn[:, k]) for k in range(4)]
                         + [(ones_row, stream_bias("b_attn"))])
                x1 = enc_sb.tile([P, D], f32)
                nc.vector.tensor_tensor(x1, h_ln, ps, ALU.add)
                h1e = enc_sb.tile([P, D], f32)
                layernorm(x1, h1e, gbe, 2, 3)

                # ff: emb = LN2(h1e + gelu(h1e@W1+b1)@W2+b2), W1/W2 streamed
                h1eT = enc_sb.tile([P, 4, P], f32r)
                transpose_into(h1eT, h1e, 4, "h1eT")
                ffh = enc_sb.tile([P, HFF], f32)
                b1 = stream_bias("ff_b1")
                for nb in range(4):
                    wc = encs.tile([P, 4, D], f32r, name="ffw", tag="ffw")
                    nc.sync.dma_start(wc, kmaj("ff_w1")[:, :, nb * D:(nb + 1) * D])
                    psf = enc_ps.tile([P, D], f32, name="epsf", tag="emm")
                    mm_group(psf, [(h1eT[:, k], wc[:, k]) for k in range(4)]
                             + [(ones_row, b1[:, nb * D:(nb + 1) * D])])
                    nc.scalar.activation(ffh[:, nb * D:(nb + 1) * D], psf, AF.Gelu)
                ffhT = enc_sb.tile([P, 16, P], f32r)
                transpose_into(ffhT, ffh, 16, "ffhT")
                ps = enc_ps.tile([P, D], f32, name="eps2", tag="emm")
                for kb in range(4):
                    wc = encs.tile([P, 4, D], f32r, name="ffw2", tag="ffw")
                    nc.sync.dma_start(wc, kmaj("ff_w2")[:, kb * 4:(kb + 1) * 4, :])
                    for k in range(4):
                        nc.tensor.matmul(ps, ffhT[:, kb * 4 + k], wc[:, k],
                                         start=(kb == 0 and k == 0), stop=False)
                nc.tensor.matmul(ps, ones_row, stream_bias("ff_b2"),
                                 start=False, stop=True)
                x2 = enc_sb.tile([P, D], f32)
                nc.vector.tensor_tensor(x2, h1e, ps, ALU.add)
                layernorm(x2, emb, gbe, 4, 5)
                transpose_into(embT, emb, 4, "embT")

                # ---------------- Gating ----------------
                psg = enc_ps.tile([P, G], f32, name="epsg", tag="emm")
                mm_group(psg, [(embT[:, k], w_g1[:, k]) for k in range(4)]
                         + [(ones_row, rb("gate_b1"))])
                g1 = enc_sb.tile([P, G], f32)
                nc.scalar.activation(g1, psg, AF.Gelu)
                g1T = enc_sb.tile([P, 2, P], f32r)
                transpose_into(g1T, g1, 2, "g1T")
                psl = enc_ps.tile([P, E], f32, name="epsl", tag="emm")
                mm_group(psl, [(g1T[:, k], w_g2[:, k]) for k in range(2)]
                         + [(ones_row, rb("gate_b2"))])

                # softmax over E=8 (free axis)
                mx = work.tile([P, 1], f32, name="smx", tag="smx")
                nc.vector.reduce_max(mx, psl, axis=AX.X)
                sh = enc_sb.tile([P, E], f32)
                nc.vector.tensor_scalar(sh, psl, mx, None, op0=ALU.subtract)
                ex = enc_sb.tile([P, E], f32)
                nc.scalar.activation(ex, sh, AF.Exp)
                sm = work.tile([P, 1], f32, name="ssm", tag="ssm")
                nc.vector.reduce_sum(sm, ex, axis=AX.X)
                rs = work.tile([P, 1], f32, name="srs", tag="srs")
                nc.vector.reciprocal(rs, sm)
                probs = enc_sb.tile([P, E], f32)
                nc.vector.tensor_scalar_mul(probs, ex, rs)
                nc.sync.dma_start(probs_d.ap(), probs)

                # usage partial: column sums of probs via ones-column matmul
                ones_col = enc_sb.tile([P, 1], f32r)
                nc.sync.dma_start(ones_col, dram["ones_col"].ap().bitcast(f32r))
                probs_r = enc_sb.tile([P, E], f32r)
                nc.vector.tensor_copy(probs_r, probs)
                psu = enc_ps.tile([1, E], f32, name="epsu", tag="emm")
                nc.tensor.matmul(psu, ones_col, probs_r, start=True, stop=True)
                usage_sb = enc_sb.tile([1, E], f32)
                nc.vector.tensor_copy(usage_sb, psu)
                nc.sync.dma_start(usage_d.ap(), usage_sb)

                # top-2 combine weights cw[b, e] = p_e * in_top2 / (p1 + p2)
                m1 = work.tile([P, 1], f32, name="m1", tag="m1")
                nc.vector.reduce_max(m1, probs, axis=AX.X)
                mask1 = enc_sb.tile([P, E], f32)
                nc.vector.tensor_scalar(mask1, probs, m1, None, op0=ALU.is_equal)
                inv1 = enc_sb.tile([P, E], f32)
                nc.vector.tensor_scalar(inv1, mask1, -1.0, 1.0,
                                        op0=ALU.mult, op1=ALU.add)
                probs2 = enc_sb.tile([P, E], f32)
                nc.vector.tensor_mul(probs2, probs, inv1)
                m2 = work.tile([P, 1], f32, name="m2", tag="m2")
                nc.vector.reduce_max(m2, probs2, axis=AX.X)
                mask2 = enc_sb.tile([P, E], f32)
                nc.vector.tensor_scalar(mask2, probs2, m2, None, op0=ALU.is_equal)
                wmask = enc_sb.tile([P, E], f32)
                nc.vector.tensor_add(wmask, mask1, mask2)
                den = work.tile([P, 1], f32, name="den", tag="den")
                nc.vector.tensor_tensor(den, m1, m2, ALU.add)
                rden = work.tile([P, 1], f32, name="rden", tag="rden")
                nc.vector.reciprocal(rden, den)
                cwt = enc_sb.tile([P, E], f32)
                nc.vector.tensor_mul(cwt, probs, wmask)
                nc.vector.tensor_scalar_mul(cw, cwt, rden)

            # ------- Experts (bf16, interleaved between GRU steps) + GRU
            with (
                tc.tile_pool(name="expw", bufs=3) as expw,
                tc.tile_pool(name="exp_sb", bufs=2) as exp_sb,
                tc.tile_pool(name="exp_ps", bufs=2, space="PSUM") as exp_ps,
                tc.tile_pool(name="gruw", bufs=1) as gruw,
                tc.tile_pool(name="gru_sb", bufs=2) as gru_sb,
                tc.tile_pool(name="gru_gt", bufs=6) as gru_gt,
                tc.tile_pool(name="gru_ps", bufs=4, space="PSUM") as gru_ps,
            ):
                embT_bf = gruw.tile([P, 4, P], bf16)
                nc.vector.tensor_copy(embT_bf, embT)

                def expert_l1(e):
                    h1x = exp_sb.tile([P, HFF], f32, name="h1x", tag="h1x")
                    h1xT = exp_sb.tile([P, 16, P], bf16, name="h1xT",
                                       tag="h1xT")
                    b1e = stream_bias(f"exp_b1_{e}")
                    for nb in range(4):
                        w1c = expw.tile([P, 4, D], bf16, name="w1c", tag="ew")
                        nc.sync.dma_start(
                            w1c,
                            dram["exp_w1"].ap()[e, :, nb * D:(nb + 1) * D]
                            .rearrange("(o p) n -> p o n", p=P))
                        pse = exp_ps.tile([P, D], f32, name="pse", tag="eps")
                        mm_group(pse,
                                 [(embT_bf[:, k], w1c[:, k]) for k in range(4)]
                                 + [(ones_row, b1e[:, nb * D:(nb + 1) * D])])
                        nc.scalar.activation(h1x[:, nb * D:(nb + 1) * D],
                                             pse, AF.Gelu)
                        for c in range(4):
                            cc = nb * 4 + c
                            pt = tp_ps.tile([P, P], f32, name="tp", tag="tp")
                            nc.tensor.transpose(
                                pt, h1x[:, cc * P:(cc + 1) * P], ident)
                            nc.vector.tensor_copy(h1xT[:, cc], pt)
                    return h1xT

                def expert_l2(e, h1xT):
                    pso = exp_ps.tile([P, D], f32, name="pso", tag="eps")
                    for kb in range(4):
                        w2c = expw.tile([P, 4, D], bf16, name="w2c", tag="ew")
                        nc.sync.dma_start(
                            w2c,
                            dram["exp_w2"].ap()[e, kb * D:(kb + 1) * D, :]
                            .rearrange("(o p) n -> p o n", p=P))
                        for k in range(4):
                            nc.tensor.matmul(pso, h1xT[:, kb * 4 + k], w2c[:, k],
                                             start=(kb == 0 and k == 0),
                                             stop=False)
                    nc.tensor.matmul(pso, ones_row, stream_bias(f"exp_b2_{e}"),
                                     start=False, stop=True)
                    if e == 0:
                        nc.vector.tensor_scalar(mixed, pso, cw[:, 0:1], None,
                                                op0=ALU.mult)
                    else:
                        nc.vector.scalar_tensor_tensor(
                            mixed, pso, cw[:, e:e + 1], mixed,
                            op0=ALU.mult, op1=ALU.add)

                # ---- GRU weights + per-gate broadcast biases
                xseqT = gruw.tile([8, S, P], bf16)
                nc.sync.dma_start(xseqT, dram["xseq_aug"].ap())
                wi0a = gruw.tile([8, 3 * D], bf16)
                nc.sync.dma_start(wi0a, dram["wi0_aug"].ap())
                wh0 = gruw.tile([P, 4, 3 * D], bf16)
                nc.sync.dma_start(wh0, kmaj_bf("wh0"))
                wi1 = gruw.tile([P, 4, 3 * D], bf16)
                nc.sync.dma_start(wi1, kmaj_bf("wi1"))
                wh1 = gruw.tile([P, 4, 3 * D], bf16)
                nc.sync.dma_start(wh1, kmaj_bf("wh1"))
                # grub rows: 0 c2-r bias, 1 c2-z bias, 2 c2-inn, 3 c1-hn, 4 c2-hn
                grub = gruw.tile([P, 5, D], f32)
                nc.gpsimd.dma_start(
                    grub, bass.AP(tensor=dram["grub"], offset=0,
                                  ap=[[0, P], [D, 5], [1, D]]))

                def gru_cell(ci, gates_mms, h_sb, h1p_sb, rz_bias, i_bias,
                             h_bias):
                    """sigma-only GRU cell (tanh(v) = 2*sigmoid(2v) - 1 keeps
                    the sigmoid LUT resident on ACT). h1p_sb = h_sb + 1.
                    Returns (h_new, h_new + 1)."""
                    pss = {}
                    for gname in ("r", "z", "i", "h"):
                        ps = gru_ps.tile([P, D], f32, name=f"ps{gname}{ci}",
                                         tag="gps")
                        mm_group(ps, gates_mms[gname])
                        pss[gname] = ps
                    if rz_bias is not None:
                        rpre = gru_gt.tile([P, D], f32, name=f"rp{ci}", tag="gt")
                        nc.vector.tensor_tensor(rpre, pss["r"], rz_bias[0],
                                                ALU.add)
                        zpre = gru_gt.tile([P, D], f32, name=f"zp{ci}", tag="gt")
                        nc.vector.tensor_tensor(zpre, pss["z"], rz_bias[1],
                                                ALU.add)
                    else:
                        rpre, zpre = pss["r"], pss["z"]
                    r_sb = gru_gt.tile([P, D], f32, name=f"r{ci}", tag="gt")
                    nc.scalar.activation(r_sb, rpre, AF.Sigmoid)
                    z_sb = gru_gt.tile([P, D], f32, name=f"z{ci}", tag="gt")
                    nc.scalar.activation(z_sb, zpre, AF.Sigmoid)
                    hb = gru_gt.tile([P, D], f32, name=f"hb{ci}", tag="gt")
                    nc.vector.tensor_tensor(hb, pss["h"], h_bias, ALU.add)
                    t1 = gru_gt.tile([P, D], f32, name=f"t1{ci}", tag="gt")
                    nc.vector.tensor_mul(t1, r_sb, hb)
                    nc.vector.tensor_tensor(t1, t1, pss["i"], ALU.add)
                    if i_bias is not None:
                        nc.vector.tensor_tensor(t1, t1, i_bias, ALU.add)
                    # n = tanh(t1) = 2*s - 1 with s = sigmoid(2*t1)
                    s_sb = gru_gt.tile([P, D], f32, name=f"s{ci}", tag="gt")
                    nc.scalar.activation(s_sb, t1, AF.Sigmoid, scale=2.0)
                    # h_new = n + z*(h - n); h - n = (h + 1) - 2s
                    dd = gru_gt.tile([P, D], f32, name=f"d{ci}", tag="gt")
                    nc.vector.scalar_tensor_tensor(dd, s_sb, -2.0, h1p_sb,
                                                   op0=ALU.mult, op1=ALU.add)
                    nc.vector.tensor_mul(dd, z_sb, dd)
                    hnew = gru_sb.tile([P, D], f32, name=f"hs{ci}",
                                       tag=f"hs{ci}")
                    nc.vector.scalar_tensor_tensor(hnew, s_sb, 2.0, dd,
                                                   op0=ALU.mult, op1=ALU.add)
                    nc.vector.tensor_scalar_add(hnew, hnew, -1.0)
                    h1p_new = gru_sb.tile([P, D], f32, name=f"hp{ci}",
                                          tag=f"hp{ci}")
                    nc.vector.tensor_scalar_add(h1p_new, hnew, 1.0)
                    return hnew, h1p_new

                h1_sb = gru_sb.tile([P, D], f32, name="hs1", tag="hs1")
                nc.vector.memset(h1_sb, 0.0)
                h2_sb = gru_sb.tile([P, D], f32, name="hs2", tag="hs2")
                nc.vector.memset(h2_sb, 0.0)
                h1p_sb = gru_sb.tile([P, D], f32, name="hp1", tag="hp1")
                nc.vector.memset(h1p_sb, 1.0)
                h2p_sb = gru_sb.tile([P, D], f32, name="hp2", tag="hp2")
                nc.vector.memset(h2p_sb, 1.0)
                h1T = None
                h2T_prev = None

                for t in range(S):
                    # interleave half an expert per step so its matmuls fill
                    # PE gaps in the recurrence and its weight DMA streams
                    # behind the compute
                    if t // 2 < E:
                        if t % 2 == 0:
                            cur_h1xT = expert_l1(t // 2)
                        else:
                            expert_l2(t // 2, cur_h1xT)

                    xs = xseqT[:, t, :]
                    g1m = {
                        "r": [(xs, wi0a[:, 0:D])],
                        "z": [(xs, wi0a[:, D:2 * D])],
                        "i": [(xs, wi0a[:, 2 * D:3 * D])],
                        "h": [],
                    }
                    if h1T is not None:
                        for k in range(4):
                            g1m["r"].append((h1T[:, k], wh0[:, k, 0:D]))
                            g1m["z"].append((h1T[:, k], wh0[:, k, D:2 * D]))
                            g1m["h"].append((h1T[:, k], wh0[:, k, 2 * D:3 * D]))
                    else:
                        g1m["h"] = [(ones_row, rb("zeroD"))]
                    h1_sb, h1p_sb = gru_cell(1, g1m, h1_sb, h1p_sb, None, None, grub[:, 3])
                    h1T = gru_sb.tile([P, 4, P], bf16, name="h1T", tag="h1T")
                    transpose_into(h1T, h1_sb, 4, "h1T")

                    g2m = {
                        "r": [(h1T[:, k], wi1[:, k, 0:D]) for k in range(4)],
                        "z": [(h1T[:, k], wi1[:, k, D:2 * D]) for k in range(4)],
                        "i": [(h1T[:, k], wi1[:, k, 2 * D:3 * D])
                              for k in range(4)],
                        "h": [],
                    }
                    if h2T_prev is not None:
                        for k in range(4):
                            g2m["r"].append((h2T_prev[:, k], wh1[:, k, 0:D]))
                            g2m["z"].append((h2T_prev[:, k],
                                             wh1[:, k, D:2 * D]))
                            g2m["h"].append((h2T_prev[:, k],
                                             wh1[:, k, 2 * D:3 * D]))
                    else:
                        g2m["h"] = [(ones_row, rb("zeroD"))]
                    h2_sb, h2p_sb = gru_cell(2, g2m, h2_sb, h2p_sb,
                                             (grub[:, 0], grub[:, 1]),
                                             grub[:, 2], grub[:, 4])
                    h2T = gru_sb.tile([P, 4, P], bf16, name="h2T", tag="h2T")
                    transpose_into(h2T, h2_sb, 4, "h2T")
                    nc.sync.dma_start(h2T_dram[t], h2T)
                    h2T_prev = h2T

            # ---------------- context = gelu(LN(mixed@mix_w + mix_b)) + emb
            ctx = persist.tile([P, D], f32)
            ctxT = persist.tile([P, D // P, P], bf16)
            with (
                tc.tile_pool(name="mixw", bufs=1) as mixw,
                tc.tile_pool(name="mix_ps", bufs=2, space="PSUM") as mix_ps,
                tc.tile_pool(name="mix_sb", bufs=1) as mix_sb,
            ):
                gbm = mixw.tile([P, 2, D], f32)
                gb_bcast(gbm, 6, 2)
                w_mix = mixw.tile([P, 4, D], f32r)
                nc.sync.dma_start(w_mix, kmaj("mix_w"))
                mixT = mix_sb.tile([P, 4, P], f32r)
                transpose_into(mixT, mixed, 4, "mixT")
                ps = mix_ps.tile([P, D], f32, name="mps", tag="mmm")
                mm_group(ps, [(mixT[:, k], w_mix[:, k]) for k in range(4)]
                         + [(ones_row, stream_bias("mix_b"))])
                mx_sb = mix_sb.tile([P, D], f32)
                nc.scalar.activation(mx_sb, ps, AF.Copy)
                mxn = mix_sb.tile([P, D], f32)
                layernorm(mx_sb, mxn, gbm, 0, 1)
                mxg = mix_sb.tile([P, D], f32)
                nc.scalar.activation(mxg, mxn, AF.Gelu)
                nc.vector.tensor_tensor(ctx, mxg, emb, ALU.add)
                transpose_into(ctxT, ctx, 4, "ctxT")

            # ---------------- decoder heads, staged so each ACT function
            # runs as one homogeneous batch (one LUT load per function)
            with (
                tc.tile_pool(name="combw", bufs=1) as combw,
                tc.tile_pool(name="cmb_ps", bufs=4, space="PSUM") as cmb_ps,
                tc.tile_pool(name="cmb_sb", bufs=3) as cmb_sb,
            ):
                gbc = combw.tile([P, 2, D], f32)
                gb_bcast(gbc, 8, 2)
                w_comb = combw.tile([P, 8, D], bf16)
                nc.sync.dma_start(w_comb, kmaj_bf("comb_w"))
                w_head = combw.tile([P, 4, OUT_F + 1], bf16)
                nc.sync.dma_start(w_head, kmaj_bf("head_w"))
                cm_all = combw.tile([P, S, D], f32)
                rooms_raw = combw.tile([P, S, OUT_F + 1], f32)

                h2Tts = []
                for t in range(S):
                    h2Tt = cmb_sb.tile([P, 4, P], bf16, name="h2Tt",
                                       tag=f"h2Tt{t % 4}")
                    nc.sync.dma_start(h2Tt, h2T_dram[t])
                    h2Tts.append(h2Tt)
                    ps = cmb_ps.tile([P, D], f32, name="cps", tag="cmm")
                    mm_group(ps,
                             [(ctxT[:, k], w_comb[:, k]) for k in range(4)]
                             + [(h2Tt[:, k], w_comb[:, 4 + k])
                                for k in range(4)]
                             + [(ones_row, rb("comb_b"))])
                    nc.vector.tensor_copy(cm_all[:, t], ps)
                for t in range(S):
                    layernorm(cm_all[:, t], cm_all[:, t], gbc, 0, 1)
                for t in range(S):
                    nc.scalar.activation(cm_all[:, t], cm_all[:, t], AF.Gelu)
                for t in range(S):
                    hidT = cmb_sb.tile([P, 4, P], bf16, name="hidT",
                                       tag="hidT")
                    transpose_into(hidT, cm_all[:, t], 4, "hidT")
                    psh = cmb_ps.tile([P, OUT_F + 1], f32, name="hps",
                                      tag="cmm")
                    mm_group(psh,
                             [(hidT[:, k], w_head[:, k]) for k in range(4)]
                             + [(ones_row, rb("head_b"))])
                    nc.vector.tensor_copy(rooms_raw[:, t], psh)
                nc.scalar.activation(rooms_raw[:, :, 0:4], rooms_raw[:, :, 0:4],
                                     AF.Sigmoid)
                nc.scalar.activation(rooms_raw[:, :, 20:22],
                                     rooms_raw[:, :, 20:22], AF.Sigmoid)
                nc.sync.dma_start(rooms_d.ap(), rooms_raw[:, :, 0:OUT_F])

    _legalize_waits(nc)
    return nc


_BUILD_CACHE = {}
LAST_EXEC_NS = None


def _make_blobs(p):
    res = _Blob()
    res.add("ones", np.ones(P))
    res.add("zeroD", np.zeros(D))
    res.add("gate_b1", p["gate_b1"])
    res.add("gate_b2", p["gate_b2"])
    res.add("bh0_n", p["gru_bh0"][2 * D:])
    res.add("bi1bh1_rz", (p["gru_bi1"] + p["gru_bh1"])[:2 * D])
    res.add("bi1_n", p["gru_bi1"][2 * D:])
    res.add("bh1_n", p["gru_bh1"][2 * D:])
    res.add("comb_b", p["comb_b"])
    res.add("head_b", np.concatenate(
        [p["coord_b"], p["type_b"], p["zone_b"], p["ext_b"], p["stop_b"],
         np.zeros(1, np.float32)]))

    st = _Blob()
    st.add("proj_b", p["proj_b"])
    st.add("b_attn", p["attn_bv"] @ p["attn_wo"] + p["attn_bo"])
    st.add("ff_b1", p["ff_b1"])
    st.add("ff_b2", p["ff_b2"])
    st.add("mix_b", p["mix_b"])
    for e in range(E):
        st.add(f"exp_b1_{e}", p["exp_b1"][e])
        st.add(f"exp_b2_{e}", p["exp_b2"][e])
    return res, st


def kernel(constraints, teacher_rooms, params, num_rooms):
    constraints = np.ascontiguousarray(np.asarray(constraints, np.float32))
    teacher_rooms = np.ascontiguousarray(np.asarray(teacher_rooms, np.float32))
    p = {k: np.ascontiguousarray(np.asarray(v, np.float32))
         for k, v in params.items()}
    S = int(num_rooms)
    B = constraints.shape[0]
    Bl = B // N_CORES
    import ml_dtypes
    bfl = ml_dtypes.bfloat16

    res, st = _make_blobs(p)

    gb = np.ascontiguousarray(np.stack(
        [p["ln0_g"], p["ln0_b"], p["ln1_g"], p["ln1_b"],
         p["ln2_g"], p["ln2_b"], p["mix_ln_g"], p["mix_ln_b"],
         p["comb_ln_g"], p["comb_ln_b"]]))

    proj_w_pad = np.zeros((P, D), np.float32)
    proj_w_pad[:F_IN] = p["proj_w"]
    attn_w = np.ascontiguousarray(p["attn_wv"] @ p["attn_wo"])

    wi0_aug = np.zeros((8, 3 * D), np.float32)
    wi0_aug[:RF] = p["gru_wi0"]
    wi0_aug[RF, :2 * D] = (p["gru_bi0"] + p["gru_bh0"])[:2 * D]
    wi0_aug[RF, 2 * D:] = p["gru_bi0"][2 * D:]

    head_w = np.ascontiguousarray(np.concatenate(
        [p["coord_w"], p["type_w"], p["zone_w"], p["ext_w"], p["stop_w"],
         np.zeros((D, 1), np.float32)], 1))

    rzsum = (p["gru_bi1"] + p["gru_bh1"])
    grub = np.ascontiguousarray(np.stack(
        [rzsum[0:D], rzsum[D:2 * D], p["gru_bi1"][2 * D:],
         p["gru_bh0"][2 * D:], p["gru_bh1"][2 * D:]]))

    key = S
    if key not in _BUILD_CACHE:
        _BUILD_CACHE[key] = _build(S, res.offs, res.total, st.offs, st.total)
    nc = _BUILD_CACHE[key]

    shared = {
        "ident": np.eye(P, dtype=np.float32),
        "ones_col": np.ones((P, 1), np.float32),
        "resblob": np.ascontiguousarray(res.array()),
        "strblob": np.ascontiguousarray(st.array()),
        "gb": gb,
        "proj_w": proj_w_pad,
        "attn_w": attn_w,
        "ff_w1": p["ff_w1"], "ff_w2": p["ff_w2"],
        "gate_w1": p["gate_w1"], "gate_w2": p["gate_w2"],
        "exp_w1": np.ascontiguousarray(p["exp_w1"].astype(bfl)),
        "exp_w2": np.ascontiguousarray(p["exp_w2"].astype(bfl)),
        "grub": grub,
        "mix_w": p["mix_w"],
        "wi0_aug": np.ascontiguousarray(wi0_aug.astype(bfl)),
        "wh0": np.ascontiguousarray(p["gru_wh0"].astype(bfl)),
        "wi1": np.ascontiguousarray(p["gru_wi1"].astype(bfl)),
        "wh1": np.ascontiguousarray(p["gru_wh1"].astype(bfl)),
        "comb_w": np.ascontiguousarray(p["comb_w"].astype(bfl)),
        "head_w": np.ascontiguousarray(head_w.astype(bfl)),
    }

    in_maps = []
    for i in range(N_CORES):
        sl = slice(i * Bl, (i + 1) * Bl)
        xseq_aug = np.zeros((8, S, P), np.float32)
        tr = teacher_rooms[sl]
        for t in range(1, S):
            xseq_aug[:RF, t, :] = tr[:, t - 1, :].T
        xseq_aug[RF, :, :] = 1.0
        m = dict(shared)
        m["constraints"] = np.ascontiguousarray(constraints[sl])
        m["xseq_aug"] = xseq_aug.astype(bfl)
        in_maps.append(m)

    global LAST_EXEC_NS
    profile = bool(os.environ.get("KERNEL_PROFILE"))
    res_hw = run_bass_kernel_spmd(nc, in_maps, core_ids=list(range(N_CORES)),
                                  trace=profile)
    LAST_EXEC_NS = res_hw.exec_time_ns

    rooms = np.concatenate([r["rooms"] for r in res_hw.results], 0)
    gate_probs = np.concatenate([r["gate_probs"] for r in res_hw.results], 0)
    usage = sum(r["usage_p"][0] for r in res_hw.results) / B
    aux = np.float32(np.mean((usage - 1.0 / E) ** 2) * E)
    return rooms, gate_probs, aux


# revision 13
# speedup vs baseline: 1.4112x; 1.0326x over previous
"""Trainium2 Bass kernel for nn_BuildifyMOE (moe_routing).

Strategy: data-parallel over the batch across 8 NeuronCores (128 tokens/core,
exactly one SBUF partition tile). Token-major layout ([tokens, features]).
All matmuls run as float32r (full PE rate at moving-dim >= 256, ~1e-4 rel
err) with PE-transposed activations as the stationary operand. Biases enter
PSUM via rank-1 (K=1) ones-row matmuls; one-shot bias vectors are streamed
from HBM since a [1, N] SBUF tile reserves N*4 bytes on every partition.
Dense expert compute; expert weights stream from HBM and overlap the
sequential GRU recurrence. Per-step h2^T tiles bounce through DRAM for the
decoder-head phase. The aux_loss batch reduction finishes on host from
per-core partial sums.
"""
import os

import numpy as np

import concourse.bass as bass
import concourse.mybir as mybir
import concourse.tile as tile
from concourse.bass_utils import run_bass_kernel_spmd

P = 128
F_IN = 64
D = 512
HFF = 2048
E = 8
G = 256
T_TYPES = 16
RF = 7
OUT_F = 4 + T_TYPES + 3  # 23
N_CORES = 8

f32 = mybir.dt.float32
f32r = mybir.dt.float32r
bf16 = mybir.dt.bfloat16
AF = mybir.ActivationFunctionType
ALU = mybir.AluOpType
AX = mybir.AxisListType


def _legalize_waits(nc):
    """This container's walrus rejects >1 sync-wait command per instruction.
    Move excess waits onto NoOp carriers inserted before, same engine."""
    n = 0
    for f in nc.m.functions:
        for blk in f.blocks:
            insts = list(blk.instructions)
            out = []
            changed = False
            for inst in insts:
                si = getattr(inst, "sync_info", None)
                waits = list(si.on_wait) if (si is not None and si.on_wait) else []
                if len(waits) > 1:
                    for w in waits[:-1]:
                        nop = mybir.InstNoOp(name=f"I-waitnop-{n}", ins=[], outs=[])
                        n += 1
                        nop.engine = inst.engine
                        nop.sync_info = mybir.SyncInfo(on_wait=[w], on_update=[])
                        out.append(nop)
                    si.on_wait = [waits[-1]]
                    changed = True
                out.append(inst)
            if changed:
                blk.instructions = out
    return n


class _Blob:
    """Accumulates 1-row fp32 constants into one [1, total] array."""

    def __init__(self):
        self.vals = []
        self.offs = {}
        self.total = 0

    def add(self, name, vec):
        vec = np.asarray(vec, np.float32).reshape(-1)
        self.offs[name] = (self.total, vec.size)
        self.vals.append(vec)
        self.total += vec.size

    def array(self):
        return np.concatenate(self.vals)[None, :]


def _build(S, res_offs, res_total, str_offs, str_total):
    nc = bass.Bass()

    dram = {}

    def din(name, shape):
        dram[name] = nc.dram_tensor(name, list(shape), f32, kind="ExternalInput")
        return dram[name]

    din("constraints", [P, F_IN])
    dram["xseq_aug"] = nc.dram_tensor("xseq_aug", [8, S, P], bf16, kind="ExternalInput")          # rows 0-6: x_seq[t].T, row 7: ones
    din("ident", [P, P])
    din("ones_col", [P, 1])
    din("resblob", [1, res_total])      # resident biases (GRU/gate/comb/head)
    din("strblob", [1, str_total])      # streamed one-shot biases
    din("gb", [10, D])                  # gamma/beta rows for the 5 LNs
    din("proj_w", [P, D])               # zero-padded 64->128 on host
    din("attn_w", [D, D])               # wv @ wo folded on host
    din("ff_w1", [D, HFF])
    din("ff_w2", [HFF, D])
    din("gate_w1", [D, G])
    din("gate_w2", [G, E])
    dram["exp_w1"] = nc.dram_tensor("exp_w1", [E, D, HFF], bf16,
                                    kind="ExternalInput")
    dram["exp_w2"] = nc.dram_tensor("exp_w2", [E, HFF, D], bf16,
                                    kind="ExternalInput")
    din("mix_w", [D, D])
    for nm, shp in [("wi0_aug", [8, 3 * D]), ("wh0", [D, 3 * D]),
                    ("wi1", [D, 3 * D]), ("wh1", [D, 3 * D])]:
        dram[nm] = nc.dram_tensor(nm, shp, bf16, kind="ExternalInput")
    dram["comb_w"] = nc.dram_tensor("comb_w", [2 * D, D], bf16,
                                    kind="ExternalInput")
    dram["head_w"] = nc.dram_tensor("head_w", [D, OUT_F + 1], bf16,
                                    kind="ExternalInput")
    din("grub", [5, D])

    rooms_d = nc.dram_tensor("rooms", [P, S, OUT_F], f32, kind="ExternalOutput")
    probs_d = nc.dram_tensor("gate_probs", [P, E], f32, kind="ExternalOutput")
    usage_d = nc.dram_tensor("usage_p", [1, E], f32, kind="ExternalOutput")

    def kmaj(name):
        """DRAM [K, N] -> fp32r AP [128, K/128, N] for SBUF K-chunk layout."""
        ap = dram[name].ap().bitcast(f32r)
        return ap.rearrange("(o p) n -> p o n", p=P)

    def kmaj_bf(name):
        return dram[name].ap().rearrange("(o p) n -> p o n", p=P)

    def gb_bcast(dst, row0, nrows):
        """Broadcast gb rows [row0:row0+nrows] across partitions into dst."""
        src = bass.AP(tensor=dram["gb"], offset=row0 * D,
                      ap=[[0, P], [D, nrows], [1, D]])
        nc.gpsimd.dma_start(dst, src)

    with tile.TileContext(nc) as tc:
        with (
            tc.tile_pool(name="persist", bufs=1) as persist,
            tc.tile_pool(name="work", bufs=6) as work,
            tc.tile_pool(name="bias_sb", bufs=2) as bias_sb,
            tc.tile_pool(name="tp_ps", bufs=2, space="PSUM") as tp_ps,
            tc.tile_pool(name="dram_sc", bufs=1, space="DRAM") as dram_sc,
        ):
            ident = persist.tile([P, P], f32)
            nc.sync.dma_start(ident, dram["ident"].ap())
            rblob = persist.tile([1, res_total], f32r)
            nc.sync.dma_start(rblob, dram["resblob"].ap().bitcast(f32r))
            eps_t = persist.tile([P, 1], f32)
            nc.vector.memset(eps_t, 1e-5)

            def rb(name):
                off, size = res_offs[name]
                return rblob[0:1, off:off + size]

            ones_row = rb("ones")  # [1, 128] of 1.0

            def stream_bias(name):
                """DMA a one-shot bias vector into a rotating [1, *] tile."""
                off, size = str_offs[name]
                t = bias_sb.tile([1, HFF], f32r, name="bstr", tag="bstr")
                nc.sync.dma_start(
                    t[:, :size],
                    dram["strblob"].ap().bitcast(f32r)[0:1, off:off + size])
                return t[0:1, 0:size]

            def mm_group(ps, pairs):
                """Emit one PSUM accumulation group from (lhsT, rhs) pairs."""
                for i, (l, r) in enumerate(pairs):
                    nc.tensor.matmul(ps, l, r, start=(i == 0),
                                     stop=(i == len(pairs) - 1))

            def transpose_into(dstT, src_sb, n_chunks, tag):
                """src_sb [128, n*128] fp32 -> dstT [128, n, 128] fp32r."""
                for c in range(n_chunks):
                    pt = tp_ps.tile([P, P], f32, name="tp", tag="tp")
                    nc.tensor.transpose(pt, src_sb[:, c * P:(c + 1) * P], ident)
                    nc.vector.tensor_copy(dstT[:, c], pt)

            def layernorm(x_sb, out_sb, gbt, gi, bi):
                """out = LN(x) * gbt[:, gi] + gbt[:, bi]."""
                st = work.tile([P, 6], f32, name="lnst", tag="lnst")
                nc.vector.bn_stats(st, x_sb)
                mv = work.tile([P, 2], f32, name="lnmv", tag="lnmv")
                nc.vector.bn_aggr(mv, st)
                rstd = work.tile([P, 1], f32, name="lnrs", tag="lnrs")
                nc.scalar.activation(rstd, mv[:, 1:2], AF.Sqrt, bias=eps_t)
                nc.vector.reciprocal(rstd, rstd)
                nc.vector.tensor_scalar(out_sb, x_sb, mv[:, 0:1], rstd,
                                        op0=ALU.subtract, op1=ALU.mult)
                nc.vector.tensor_mul(out_sb, out_sb, gbt[:, gi])
                nc.vector.tensor_add(out_sb, out_sb, gbt[:, bi])

            # persistent activations
            emb = persist.tile([P, D], f32)
            embT = persist.tile([P, D // P, P], f32r)
            cw = persist.tile([P, E], f32)
            mixed = persist.tile([P, D], f32)
            h2T_dram = dram_sc.tile([S, P, 4, P], bf16)

            # ---------------- Encoder + gating ----------------
            with (
                tc.tile_pool(name="encw", bufs=1) as encw,
                tc.tile_pool(name="encs", bufs=2) as encs,
                tc.tile_pool(name="enc_ps", bufs=2, space="PSUM") as enc_ps,
                tc.tile_pool(name="enc_sb", bufs=1) as enc_sb,
            ):
                gbe = encw.tile([P, 6, D], f32)
                gb_bcast(gbe, 0, 6)
                w_proj = encw.tile([P, D], f32r)
                nc.sync.dma_start(w_proj, dram["proj_w"].ap().bitcast(f32r))
                w_attn = encw.tile([P, 4, D], f32r)
                nc.sync.dma_start(w_attn, kmaj("attn_w"))
                w_g1 = encw.tile([P, 4, G], f32r)
                nc.sync.dma_start(w_g1, kmaj("gate_w1"))
                w_g2 = encw.tile([P, 2, E], f32r)
                nc.sync.dma_start(w_g2, kmaj("gate_w2"))

                cons = enc_sb.tile([P, P], f32)
                nc.vector.memset(cons, 0.0)
                nc.sync.dma_start(cons[:, 0:F_IN], dram["constraints"].ap())
                consT = enc_sb.tile([P, P], f32r)
                pt0 = tp_ps.tile([P, P], f32, name="tp", tag="tp")
                nc.tensor.transpose(pt0, cons, ident)
                nc.vector.tensor_copy(consT, pt0)

                # h = LN0(cons @ proj_w + proj_b)
                ps = enc_ps.tile([P, D], f32, name="eps0", tag="emm")
                mm_group(ps, [(consT, w_proj),
                              (ones_row, stream_bias("proj_b"))])
                h_pre = enc_sb.tile([P, D], f32)
                nc.scalar.activation(h_pre, ps, AF.Copy)
                h_ln = enc_sb.tile([P, D], f32)
                layernorm(h_pre, h_ln, gbe, 0, 1)

                # attn (folded): x1 = h_ln + h_ln @ attn_w + b_attn; h1e = LN1
                hT = enc_sb.tile([P, 4, P], f32r)
                transpose_into(hT, h_ln, 4, "hT")
                ps = enc_ps.tile([P, D], f32, name="eps1", tag="emm")
                mm_group(ps, [(hT[:, k], w_attn[:, k]) for k in range(4)]
                         + [(ones_row, stream_bias("b_attn"))])
                x1 = enc_sb.tile([P, D], f32)
                nc.vector.tensor_tensor(x1, h_ln, ps, ALU.add)
                h1e = enc_sb.tile([P, D], f32)
                layernorm(x1, h1e, gbe, 2, 3)

                # ff: emb = LN2(h1e + gelu(h1e@W1+b1)@W2+b2), W1/W2 streamed
                h1eT = enc_sb.tile([P, 4, P], f32r)
                transpose_into(h1eT, h1e, 4, "h1eT")
                ffh = enc_sb.tile([P, HFF], f32)
                b1 = stream_bias("ff_b1")
                for nb in range(4):
                    wc = encs.tile([P, 4, D], f32r, name="ffw", tag="ffw")
                    nc.sync.dma_start(wc, kmaj("ff_w1")[:, :, nb * D:(nb + 1) * D])
                    psf = enc_ps.tile([P, D], f32, name="epsf", tag="emm")
                    mm_group(psf, [(h1eT[:, k], wc[:, k]) for k in range(4)]
                             + [(ones_row, b1[:, nb * D:(nb + 1) * D])])
                    nc.scalar.activation(ffh[:, nb * D:(nb + 1) * D], psf, AF.Gelu)
                ffhT = enc_sb.tile([P, 16, P], f32r)
                transpose_into(ffhT, ffh, 16, "ffhT")
                ps = enc_ps.tile([P, D], f32, name="eps2", tag="emm")
                for kb in range(4):
                    wc = encs.tile([P, 4, D], f32r, name="ffw2", tag="ffw")
                    nc.sync.dma_start(wc, kmaj("ff_w2")[:, kb * 4:(kb + 1) * 4, :])
                    for k in range(4):
                        nc.tensor.matmul(ps, ffhT[:, kb * 4 + k], wc[:, k],
                                         start=(kb == 0 and k == 0), stop=False)
                nc.tensor.matmul(ps, ones_row, stream_bias("ff_b2"),
                                 start=False, stop=True)
                x2 = enc_sb.tile([P, D], f32)
                nc.vector.tensor_tensor(x2, h1e, ps, ALU.add)
                layernorm(x2, emb, gbe, 4, 5)
                transpose_into(embT, emb, 4, "embT")

                # ---------------- Gating ----------------
                psg = enc_ps.tile([P, G], f32, name="epsg", tag="emm")
                mm_group(psg, [(embT[:, k], w_g1[:, k]) for k in range(4)]
                         + [(ones_row, rb("gate_b1"))])
                g1 = enc_sb.tile([P, G], f32)
                nc.scalar.activation(g1, psg, AF.Gelu)
                g1T = enc_sb.tile([P, 2, P], f32r)
                transpose_into(g1T, g1, 2, "g1T")
                psl = enc_ps.tile([P, E], f32, name="epsl", tag="emm")
                mm_group(psl, [(g1T[:, k], w_g2[:, k]) for k in range(2)]
                         + [(ones_row, rb("gate_b2"))])

                # softmax over E=8 (free axis)
                mx = work.tile([P, 1], f32, name="smx", tag="smx")
                nc.vector.reduce_max(mx, psl, axis=AX.X)
                sh = enc_sb.tile([P, E], f32)
                nc.vector.tensor_scalar(sh, psl, mx, None, op0=ALU.subtract)
                ex = enc_sb.tile([P, E], f32)
                nc.scalar.activation(ex, sh, AF.Exp)
                sm = work.tile([P, 1], f32, name="ssm", tag="ssm")
                nc.vector.reduce_sum(sm, ex, axis=AX.X)
                rs = work.tile([P, 1], f32, name="srs", tag="srs")
                nc.vector.reciprocal(rs, sm)
                probs = enc_sb.tile([P, E], f32)
                nc.vector.tensor_scalar_mul(probs, ex, rs)
                nc.sync.dma_start(probs_d.ap(), probs)

                # usage partial: column sums of probs via ones-column matmul
                ones_col = enc_sb.tile([P, 1], f32r)
                nc.sync.dma_start(ones_col, dram["ones_col"].ap().bitcast(f32r))
                probs_r = enc_sb.tile([P, E], f32r)
                nc.vector.tensor_copy(probs_r, probs)
                psu = enc_ps.tile([1, E], f32, name="epsu", tag="emm")
                nc.tensor.matmul(psu, ones_col, probs_r, start=True, stop=True)
                usage_sb = enc_sb.tile([1, E], f32)
                nc.vector.tensor_copy(usage_sb, psu)
                nc.sync.dma_start(usage_d.ap(), usage_sb)

                # top-2 combine weights cw[b, e] = p_e * in_top2 / (p1 + p2)
                m1 = work.tile([P, 1], f32, name="m1", tag="m1")
                nc.vector.reduce_max(m1, probs, axis=AX.X)
                mask1 = enc_sb.tile([P, E], f32)
                nc.vector.tensor_scalar(mask1, probs, m1, None, op0=ALU.is_equal)
                inv1 = enc_sb.tile([P, E], f32)
                nc.vector.tensor_scalar(inv1, mask1, -1.0, 1.0,
                                        op0=ALU.mult, op1=ALU.add)
                probs2 = enc_sb.tile([P, E], f32)
                nc.vector.tensor_mul(probs2, probs, inv1)
                m2 = work.tile([P, 1], f32, name="m2", tag="m2")
                nc.vector.reduce_max(m2, probs2, axis=AX.X)
                mask2 = enc_sb.tile([P, E], f32)
                nc.vector.tensor_scalar(mask2, probs2, m2, None, op0=ALU.is_equal)
                wmask = enc_sb.tile([P, E], f32)
                nc.vector.tensor_add(wmask, mask1, mask2)
                den = work.tile([P, 1], f32, name="den", tag="den")
                nc.vector.tensor_tensor(den, m1, m2, ALU.add)
                rden = work.tile([P, 1], f32, name="rden", tag="rden")
                nc.vector.reciprocal(rden, den)
                cwt = enc_sb.tile([P, E], f32)
                nc.vector.tensor_mul(cwt, probs, wmask)
                nc.vector.tensor_scalar_mul(cw, cwt, rden)

            # ------- Experts (bf16, interleaved between GRU steps) + GRU
            with (
                tc.tile_pool(name="expw", bufs=3) as expw,
                tc.tile_pool(name="exp_sb", bufs=2) as exp_sb,
                tc.tile_pool(name="exp_ps", bufs=2, space="PSUM") as exp_ps,
                tc.tile_pool(name="gruw", bufs=1) as gruw,
                tc.tile_pool(name="gru_sb", bufs=2) as gru_sb,
                tc.tile_pool(name="gru_gt", bufs=6) as gru_gt,
                tc.tile_pool(name="gru_ps", bufs=4, space="PSUM") as gru_ps,
            ):
                embT_bf = gruw.tile([P, 4, P], bf16)
                nc.vector.tensor_copy(embT_bf, embT)

                def expert_l1(e):
                    h1x = exp_sb.tile([P, HFF], f32, name="h1x", tag="h1x")
                    h1xT = exp_sb.tile([P, 16, P], bf16, name="h1xT",
                                       tag="h1xT")
                    b1e = stream_bias(f"exp_b1_{e}")
                    for nb in range(4):
                        w1c = expw.tile([P, 4, D], bf16, name="w1c", tag="ew")
                        nc.sync.dma_start(
                            w1c,
                            dram["exp_w1"].ap()[e, :, nb * D:(nb + 1) * D]
                            .rearrange("(o p) n -> p o n", p=P))
                        pse = exp_ps.tile([P, D], f32, name="pse", tag="eps")
                        mm_group(pse,
                                 [(embT_bf[:, k], w1c[:, k]) for k in range(4)]
                                 + [(ones_row, b1e[:, nb * D:(nb + 1) * D])])
                        sg = exp_sb.tile([P, D], f32, name="sg", tag="sg")
                        nc.scalar.activation(sg, pse, AF.Sigmoid, scale=1.702)
                        nc.vector.tensor_tensor(
                            h1x[:, nb * D:(nb + 1) * D], sg, pse, ALU.mult)
                        for c in range(4):
                            cc = nb * 4 + c
                            pt = tp_ps.tile([P, P], f32, name="tp", tag="tp")
                            nc.tensor.transpose(
                                pt, h1x[:, cc * P:(cc + 1) * P], ident)
                            nc.vector.tensor_copy(h1xT[:, cc], pt)
                    return h1xT

                def expert_l2(e, h1xT):
                    pso = exp_ps.tile([P, D], f32, name="pso", tag="eps")
                    for kb in range(4):
                        w2c = expw.tile([P, 4, D], bf16, name="w2c", tag="ew")
                        nc.sync.dma_start(
                            w2c,
                            dram["exp_w2"].ap()[e, kb * D:(kb + 1) * D, :]
                            .rearrange("(o p) n -> p o n", p=P))
                        for k in range(4):
                            nc.tensor.matmul(pso, h1xT[:, kb * 4 + k], w2c[:, k],
                                             start=(kb == 0 and k == 0),
                                             stop=False)
                    nc.tensor.matmul(pso, ones_row, stream_bias(f"exp_b2_{e}"),
                                     start=False, stop=True)
                    if e == 0:
                        nc.vector.tensor_scalar(mixed, pso, cw[:, 0:1], None,
                                                op0=ALU.mult)
                    else:
                        nc.vector.scalar_tensor_tensor(
                            mixed, pso, cw[:, e:e + 1], mixed,
                            op0=ALU.mult, op1=ALU.add)

                # ---- GRU weights + per-gate broadcast biases
                xseqT = gruw.tile([8, S, P], bf16)
                nc.sync.dma_start(xseqT, dram["xseq_aug"].ap())
                wi0a = gruw.tile([8, 3 * D], bf16)
                nc.sync.dma_start(wi0a, dram["wi0_aug"].ap())
                wh0 = gruw.tile([P, 4, 3 * D], bf16)
                nc.sync.dma_start(wh0, kmaj_bf("wh0"))
                wi1 = gruw.tile([P, 4, 3 * D], bf16)
                nc.sync.dma_start(wi1, kmaj_bf("wi1"))
                wh1 = gruw.tile([P, 4, 3 * D], bf16)
                nc.sync.dma_start(wh1, kmaj_bf("wh1"))
                # grub rows: 0 c2-r bias, 1 c2-z bias, 2 c2-inn, 3 c1-hn, 4 c2-hn
                grub = gruw.tile([P, 5, D], f32)
                nc.gpsimd.dma_start(
                    grub, bass.AP(tensor=dram["grub"], offset=0,
                                  ap=[[0, P], [D, 5], [1, D]]))

                def gru_cell(ci, gates_mms, h_sb, h1p_sb, rz_bias, i_bias,
                             h_bias):
                    """sigma-only GRU cell (tanh(v) = 2*sigmoid(2v) - 1 keeps
                    the sigmoid LUT resident on ACT). h1p_sb = h_sb + 1.
                    Returns (h_new, h_new + 1)."""
                    pss = {}
                    for gname in ("r", "z", "i", "h"):
                        ps = gru_ps.tile([P, D], f32, name=f"ps{gname}{ci}",
                                         tag="gps")
                        mm_group(ps, gates_mms[gname])
                        pss[gname] = ps
                    if rz_bias is not None:
                        rpre = gru_gt.tile([P, D], f32, name=f"rp{ci}", tag="gt")
                        nc.vector.tensor_tensor(rpre, pss["r"], rz_bias[0],
                                                ALU.add)
                        zpre = gru_gt.tile([P, D], f32, name=f"zp{ci}", tag="gt")
                        nc.vector.tensor_tensor(zpre, pss["z"], rz_bias[1],
                                                ALU.add)
                    else:
                        rpre, zpre = pss["r"], pss["z"]
                    r_sb = gru_gt.tile([P, D], f32, name=f"r{ci}", tag="gt")
                    nc.scalar.activation(r_sb, rpre, AF.Sigmoid)
                    z_sb = gru_gt.tile([P, D], f32, name=f"z{ci}", tag="gt")
                    nc.scalar.activation(z_sb, zpre, AF.Sigmoid)
                    hb = gru_gt.tile([P, D], f32, name=f"hb{ci}", tag="gt")
                    nc.vector.tensor_tensor(hb, pss["h"], h_bias, ALU.add)
                    t1 = gru_gt.tile([P, D], f32, name=f"t1{ci}", tag="gt")
                    nc.vector.tensor_mul(t1, r_sb, hb)
                    nc.vector.tensor_tensor(t1, t1, pss["i"], ALU.add)
                    if i_bias is not None:
                        nc.vector.tensor_tensor(t1, t1, i_bias, ALU.add)
                    # n = tanh(t1) = 2*s - 1 with s = sigmoid(2*t1)
                    s_sb = gru_gt.tile([P, D], f32, name=f"s{ci}", tag="gt")
                    nc.scalar.activation(s_sb, t1, AF.Sigmoid, scale=2.0)
                    # h_new = n + z*(h - n); h - n = (h + 1) - 2s
                    dd = gru_gt.tile([P, D], f32, name=f"d{ci}", tag="gt")
                    nc.vector.scalar_tensor_tensor(dd, s_sb, -2.0, h1p_sb,
                                                   op0=ALU.mult, op1=ALU.add)
                    nc.vector.tensor_mul(dd, z_sb, dd)
                    hnew = gru_sb.tile([P, D], f32, name=f"hs{ci}",
                                       tag=f"hs{ci}")
                    nc.vector.scalar_tensor_tensor(hnew, s_sb, 2.0, dd,
                                                   op0=ALU.mult, op1=ALU.add)
                    nc.vector.tensor_scalar_add(hnew, hnew, -1.0)
                    h1p_new = gru_sb.tile([P, D], f32, name=f"hp{ci}",
                                          tag=f"hp{ci}")
                    nc.vector.tensor_scalar_add(h1p_new, hnew, 1.0)
                    return hnew, h1p_new

                h1_sb = gru_sb.tile([P, D], f32, name="hs1", tag="hs1")
                nc.vector.memset(h1_sb, 0.0)
                h2_sb = gru_sb.tile([P, D], f32, name="hs2", tag="hs2")
                nc.vector.memset(h2_sb, 0.0)
                h1p_sb = gru_sb.tile([P, D], f32, name="hp1", tag="hp1")
                nc.vector.memset(h1p_sb, 1.0)
                h2p_sb = gru_sb.tile([P, D], f32, name="hp2", tag="hp2")
                nc.vector.memset(h2p_sb, 1.0)
                h1T = None
                h2T_prev = None

                for t in range(S):
                    # interleave half an expert per step so its matmuls fill
                    # PE gaps in the recurrence and its weight DMA streams
                    # behind the compute
                    if t // 2 < E:
                        if t % 2 == 0:
                            cur_h1xT = expert_l1(t // 2)
                        else:
                            expert_l2(t // 2, cur_h1xT)

                    xs = xseqT[:, t, :]
                    g1m = {
                        "r": [(xs, wi0a[:, 0:D])],
                        "z": [(xs, wi0a[:, D:2 * D])],
                        "i": [(xs, wi0a[:, 2 * D:3 * D])],
                        "h": [],
                    }
                    if h1T is not None:
                        for k in range(4):
                            g1m["r"].append((h1T[:, k], wh0[:, k, 0:D]))
                            g1m["z"].append((h1T[:, k], wh0[:, k, D:2 * D]))
                            g1m["h"].append((h1T[:, k], wh0[:, k, 2 * D:3 * D]))
                    else:
                        g1m["h"] = [(ones_row, rb("zeroD"))]
                    h1_sb, h1p_sb = gru_cell(1, g1m, h1_sb, h1p_sb, None, None, grub[:, 3])
                    h1T = gru_sb.tile([P, 4, P], bf16, name="h1T", tag="h1T")
                    transpose_into(h1T, h1_sb, 4, "h1T")

                    g2m = {"r": [], "z": [], "i": [], "h": []}
                    if h2T_prev is not None:
                        for k in range(4):
                            g2m["r"].append((h2T_prev[:, k], wh1[:, k, 0:D]))
                            g2m["z"].append((h2T_prev[:, k],
                                             wh1[:, k, D:2 * D]))
                            g2m["h"].append((h2T_prev[:, k],
                                             wh1[:, k, 2 * D:3 * D]))
                    else:
                        g2m["h"] = [(ones_row, rb("zeroD"))]
                    for k in range(4):
                        g2m["r"].append((h1T[:, k], wi1[:, k, 0:D]))
                        g2m["z"].append((h1T[:, k], wi1[:, k, D:2 * D]))
                        g2m["i"].append((h1T[:, k], wi1[:, k, 2 * D:3 * D]))
                    h2_sb, h2p_sb = gru_cell(2, g2m, h2_sb, h2p_sb,
                                             (grub[:, 0], grub[:, 1]),
                                             grub[:, 2], grub[:, 4])
                    h2T = gru_sb.tile([P, 4, P], bf16, name="h2T", tag="h2T")
                    transpose_into(h2T, h2_sb, 4, "h2T")
                    nc.sync.dma_start(h2T_dram[t], h2T)
                    h2T_prev = h2T

            # ---------------- context = gelu(LN(mixed@mix_w + mix_b)) + emb
            ctx = persist.tile([P, D], f32)
            ctxT = persist.tile([P, D // P, P], bf16)
            with (
                tc.tile_pool(name="mixw", bufs=1) as mixw,
                tc.tile_pool(name="mix_ps", bufs=2, space="PSUM") as mix_ps,
                tc.tile_pool(name="mix_sb", bufs=1) as mix_sb,
            ):
                gbm = mixw.tile([P, 2, D], f32)
                gb_bcast(gbm, 6, 2)
                w_mix = mixw.tile([P, 4, D], f32r)
                nc.sync.dma_start(w_mix, kmaj("mix_w"))
                mixT = mix_sb.tile([P, 4, P], f32r)
                transpose_into(mixT, mixed, 4, "mixT")
                ps = mix_ps.tile([P, D], f32, name="mps", tag="mmm")
                mm_group(ps, [(mixT[:, k], w_mix[:, k]) for k in range(4)]
                         + [(ones_row, stream_bias("mix_b"))])
                mx_sb = mix_sb.tile([P, D], f32)
                nc.scalar.activation(mx_sb, ps, AF.Copy)
                mxn = mix_sb.tile([P, D], f32)
                layernorm(mx_sb, mxn, gbm, 0, 1)
                mxg = mix_sb.tile([P, D], f32)
                nc.scalar.activation(mxg, mxn, AF.Gelu)
                nc.vector.tensor_tensor(ctx, mxg, emb, ALU.add)
                transpose_into(ctxT, ctx, 4, "ctxT")

            # ---------------- decoder heads, staged so each ACT function
            # runs as one homogeneous batch (one LUT load per function)
            with (
                tc.tile_pool(name="combw", bufs=1) as combw,
                tc.tile_pool(name="cmb_ps", bufs=4, space="PSUM") as cmb_ps,
                tc.tile_pool(name="cmb_sb", bufs=3) as cmb_sb,
            ):
                gbc = combw.tile([P, 2, D], f32)
                gb_bcast(gbc, 8, 2)
                w_comb = combw.tile([P, 8, D], bf16)
                nc.sync.dma_start(w_comb, kmaj_bf("comb_w"))
                w_head = combw.tile([P, 4, OUT_F + 1], bf16)
                nc.sync.dma_start(w_head, kmaj_bf("head_w"))
                cm_all = combw.tile([P, S, D], f32)
                rooms_raw = combw.tile([P, S, OUT_F + 1], f32)

                h2Tts = []
                for t in range(S):
                    h2Tt = cmb_sb.tile([P, 4, P], bf16, name="h2Tt",
                                       tag=f"h2Tt{t % 4}")
                    nc.sync.dma_start(h2Tt, h2T_dram[t])
                    h2Tts.append(h2Tt)
                    ps = cmb_ps.tile([P, D], f32, name="cps", tag="cmm")
                    mm_group(ps,
                             [(ctxT[:, k], w_comb[:, k]) for k in range(4)]
                             + [(h2Tt[:, k], w_comb[:, 4 + k])
                                for k in range(4)]
                             + [(ones_row, rb("comb_b"))])
                    nc.vector.tensor_copy(cm_all[:, t], ps)
                for t in range(S):
                    layernorm(cm_all[:, t], cm_all[:, t], gbc, 0, 1)
                for t in range(S):
                    nc.scalar.activation(cm_all[:, t], cm_all[:, t], AF.Gelu)
                for t in range(S):
                    hidT = cmb_sb.tile([P, 4, P], bf16, name="hidT",
                                       tag="hidT")
                    transpose_into(hidT, cm_all[:, t], 4, "hidT")
                    psh = cmb_ps.tile([P, OUT_F + 1], f32, name="hps",
                                      tag="cmm")
                    mm_group(psh,
                             [(hidT[:, k], w_head[:, k]) for k in range(4)]
                             + [(ones_row, rb("head_b"))])
                    nc.vector.tensor_copy(rooms_raw[:, t], psh)
                nc.scalar.activation(rooms_raw[:, :, 0:4], rooms_raw[:, :, 0:4],
                                     AF.Sigmoid)
                nc.scalar.activation(rooms_raw[:, :, 20:22],
                                     rooms_raw[:, :, 20:22], AF.Sigmoid)
                nc.sync.dma_start(rooms_d.ap(), rooms_raw[:, :, 0:OUT_F])

    _legalize_waits(nc)
    return nc


_BUILD_CACHE = {}
LAST_EXEC_NS = None


def _make_blobs(p):
    res = _Blob()
    res.add("ones", np.ones(P))
    res.add("zeroD", np.zeros(D))
    res.add("gate_b1", p["gate_b1"])
    res.add("gate_b2", p["gate_b2"])
    res.add("bh0_n", p["gru_bh0"][2 * D:])
    res.add("bi1bh1_rz", (p["gru_bi1"] + p["gru_bh1"])[:2 * D])
    res.add("bi1_n", p["gru_bi1"][2 * D:])
    res.add("bh1_n", p["gru_bh1"][2 * D:])
    res.add("comb_b", p["comb_b"])
    res.add("head_b", np.concatenate(
        [p["coord_b"], p["type_b"], p["zone_b"], p["ext_b"], p["stop_b"],
         np.zeros(1, np.float32)]))

    st = _Blob()
    st.add("proj_b", p["proj_b"])
    st.add("b_attn", p["attn_bv"] @ p["attn_wo"] + p["attn_bo"])
    st.add("ff_b1", p["ff_b1"])
    st.add("ff_b2", p["ff_b2"])
    st.add("mix_b", p["mix_b"])
    for e in range(E):
        st.add(f"exp_b1_{e}", p["exp_b1"][e])
        st.add(f"exp_b2_{e}", p["exp_b2"][e])
    return res, st


def kernel(constraints, teacher_rooms, params, num_rooms):
    constraints = np.ascontiguousarray(np.asarray(constraints, np.float32))
    teacher_rooms = np.ascontiguousarray(np.asarray(teacher_rooms, np.float32))
    p = {k: np.ascontiguousarray(np.asarray(v, np.float32))
         for k, v in params.items()}
    S = int(num_rooms)
    B = constraints.shape[0]
    Bl = B // N_CORES
    import ml_dtypes
    bfl = ml_dtypes.bfloat16

    res, st = _make_blobs(p)

    gb = np.ascontiguousarray(np.stack(
        [p["ln0_g"], p["ln0_b"], p["ln1_g"], p["ln1_b"],
         p["ln2_g"], p["ln2_b"], p["mix_ln_g"], p["mix_ln_b"],
         p["comb_ln_g"], p["comb_ln_b"]]))

    proj_w_pad = np.zeros((P, D), np.float32)
    proj_w_pad[:F_IN] = p["proj_w"]
    attn_w = np.ascontiguousarray(p["attn_wv"] @ p["attn_wo"])

    wi0_aug = np.zeros((8, 3 * D), np.float32)
    wi0_aug[:RF] = p["gru_wi0"]
    wi0_aug[RF, :2 * D] = (p["gru_bi0"] + p["gru_bh0"])[:2 * D]
    wi0_aug[RF, 2 * D:] = p["gru_bi0"][2 * D:]

    head_w = np.ascontiguousarray(np.concatenate(
        [p["coord_w"], p["type_w"], p["zone_w"], p["ext_w"], p["stop_w"],
         np.zeros((D, 1), np.float32)], 1))

    rzsum = (p["gru_bi1"] + p["gru_bh1"])
    grub = np.ascontiguousarray(np.stack(
        [rzsum[0:D], rzsum[D:2 * D], p["gru_bi1"][2 * D:],
         p["gru_bh0"][2 * D:], p["gru_bh1"][2 * D:]]))

    key = S
    if key not in _BUILD_CACHE:
        _BUILD_CACHE[key] = _build(S, res.offs, res.total, st.offs, st.total)
    nc = _BUILD_CACHE[key]

    shared = {
        "ident": np.eye(P, dtype=np.float32),
        "ones_col": np.ones((P, 1), np.float32),
        "resblob": np.ascontiguousarray(res.array()),
        "strblob": np.ascontiguousarray(st.array()),
        "gb": gb,
        "proj_w": proj_w_pad,
        "attn_w": attn_w,
        "ff_w1": p["ff_w1"], "ff_w2": p["ff_w2"],
        "gate_w1": p["gate_w1"], "gate_w2": p["gate_w2"],
        "exp_w1": np.ascontiguousarray(p["exp_w1"].astype(bfl)),
        "exp_w2": np.ascontiguousarray(p["exp_w2"].astype(bfl)),
        "grub": grub,
        "mix_w": p["mix_w"],
        "wi0_aug": np.ascontiguousarray(wi0_aug.astype(bfl)),
        "wh0": np.ascontiguousarray(p["gru_wh0"].astype(bfl)),
        "wi1": np.ascontiguousarray(p["gru_wi1"].astype(bfl)),
        "wh1": np.ascontiguousarray(p["gru_wh1"].astype(bfl)),
        "comb_w": np.ascontiguousarray(p["comb_w"].astype(bfl)),
        "head_w": np.ascontiguousarray(head_w.astype(bfl)),
    }

    in_maps = []
    for i in range(N_CORES):
        sl = slice(i * Bl, (i + 1) * Bl)
        xseq_aug = np.zeros((8, S, P), np.float32)
        tr = teacher_rooms[sl]
        for t in range(1, S):
            xseq_aug[:RF, t, :] = tr[:, t - 1, :].T
        xseq_aug[RF, :, :] = 1.0
        m = dict(shared)
        m["constraints"] = np.ascontiguousarray(constraints[sl])
        m["xseq_aug"] = xseq_aug.astype(bfl)
        in_maps.append(m)

    global LAST_EXEC_NS
    profile = bool(os.environ.get("KERNEL_PROFILE"))
    res_hw = run_bass_kernel_spmd(nc, in_maps, core_ids=list(range(N_CORES)),
                                  trace=profile)
    LAST_EXEC_NS = res_hw.exec_time_ns

    rooms = np.concatenate([r["rooms"] for r in res_hw.results], 0)
    gate_probs = np.concatenate([r["gate_probs"] for r in res_hw.results], 0)
    usage = sum(r["usage_p"][0] for r in res_hw.results) / B
    aux = np.float32(np.mean((usage - 1.0 / E) ** 2) * E)
    return rooms, gate_probs, aux


# revision 14
# speedup vs baseline: 1.4665x; 1.0392x over previous
"""Trainium2 Bass kernel for nn_BuildifyMOE (moe_routing).

Strategy: data-parallel over the batch across 8 NeuronCores (128 tokens/core,
exactly one SBUF partition tile). Token-major layout ([tokens, features]).
All matmuls run as float32r (full PE rate at moving-dim >= 256, ~1e-4 rel
err) with PE-transposed activations as the stationary operand. Biases enter
PSUM via rank-1 (K=1) ones-row matmuls; one-shot bias vectors are streamed
from HBM since a [1, N] SBUF tile reserves N*4 bytes on every partition.
Dense expert compute; expert weights stream from HBM and overlap the
sequential GRU recurrence. Per-step h2^T tiles bounce through DRAM for the
decoder-head phase. The aux_loss batch reduction finishes on host from
per-core partial sums.
"""
import os

import numpy as np

import concourse.bass as bass
import concourse.mybir as mybir
import concourse.tile as tile
from concourse.bass_utils import run_bass_kernel_spmd

P = 128
F_IN = 64
D = 512
HFF = 2048
E = 8
G = 256
T_TYPES = 16
RF = 7
OUT_F = 4 + T_TYPES + 3  # 23
N_CORES = 8

f32 = mybir.dt.float32
f32r = mybir.dt.float32r
bf16 = mybir.dt.bfloat16
AF = mybir.ActivationFunctionType
ALU = mybir.AluOpType
AX = mybir.AxisListType


def _legalize_waits(nc):
    """This container's walrus rejects >1 sync-wait command per instruction.
    Move excess waits onto NoOp carriers inserted before, same engine."""
    n = 0
    for f in nc.m.functions:
        for blk in f.blocks:
            insts = list(blk.instructions)
            out = []
            changed = False
            for inst in insts:
                si = getattr(inst, "sync_info", None)
                waits = list(si.on_wait) if (si is not None and si.on_wait) else []
                if len(waits) > 1:
                    for w in waits[:-1]:
                        nop = mybir.InstNoOp(name=f"I-waitnop-{n}", ins=[], outs=[])
                        n += 1
                        nop.engine = inst.engine
                        nop.sync_info = mybir.SyncInfo(on_wait=[w], on_update=[])
                        out.append(nop)
                    si.on_wait = [waits[-1]]
                    changed = True
                out.append(inst)
            if changed:
                blk.instructions = out
    return n


class _Blob:
    """Accumulates 1-row fp32 constants into one [1, total] array."""

    def __init__(self):
        self.vals = []
        self.offs = {}
        self.total = 0

    def add(self, name, vec):
        vec = np.asarray(vec, np.float32).reshape(-1)
        self.offs[name] = (self.total, vec.size)
        self.vals.append(vec)
        self.total += vec.size

    def array(self):
        return np.concatenate(self.vals)[None, :]


def _build(S, res_offs, res_total, str_offs, str_total):
    nc = bass.Bass()

    dram = {}

    def din(name, shape):
        dram[name] = nc.dram_tensor(name, list(shape), f32, kind="ExternalInput")
        return dram[name]

    din("constraints", [P, F_IN])
    dram["xseq_aug"] = nc.dram_tensor("xseq_aug", [8, S, P], bf16, kind="ExternalInput")          # rows 0-6: x_seq[t].T, row 7: ones
    din("ident", [P, P])
    din("ones_col", [P, 1])
    din("resblob", [1, res_total])      # resident biases (GRU/gate/comb/head)
    din("strblob", [1, str_total])      # streamed one-shot biases
    din("gb", [10, D])                  # gamma/beta rows for the 5 LNs
    din("proj_w", [P, D])               # zero-padded 64->128 on host
    din("attn_w", [D, D])               # wv @ wo folded on host
    din("ff_w1", [D, HFF])
    din("ff_w2", [HFF, D])
    din("gate_w1", [D, G])
    din("gate_w2", [G, E])
    dram["exp_w1"] = nc.dram_tensor("exp_w1", [E, D, HFF], bf16,
                                    kind="ExternalInput")
    dram["exp_w2"] = nc.dram_tensor("exp_w2", [E, HFF, D], bf16,
                                    kind="ExternalInput")
    din("mix_w", [D, D])
    for nm, shp in [("wi0_aug", [8, 3 * D]), ("wh0", [D, 3 * D]),
                    ("wi1", [D, 3 * D]), ("wh1", [D, 3 * D])]:
        dram[nm] = nc.dram_tensor(nm, shp, bf16, kind="ExternalInput")
    dram["comb_w"] = nc.dram_tensor("comb_w", [2 * D, D], bf16,
                                    kind="ExternalInput")
    dram["head_w"] = nc.dram_tensor("head_w", [D, OUT_F + 1], bf16,
                                    kind="ExternalInput")
    din("grub", [5, D])

    rooms_d = nc.dram_tensor("rooms", [P, S, OUT_F], f32, kind="ExternalOutput")
    probs_d = nc.dram_tensor("gate_probs", [P, E], f32, kind="ExternalOutput")
    usage_d = nc.dram_tensor("usage_p", [1, E], f32, kind="ExternalOutput")

    def kmaj(name):
        """DRAM [K, N] -> fp32r AP [128, K/128, N] for SBUF K-chunk layout."""
        ap = dram[name].ap().bitcast(f32r)
        return ap.rearrange("(o p) n -> p o n", p=P)

    def kmaj_bf(name):
        return dram[name].ap().rearrange("(o p) n -> p o n", p=P)

    def gb_bcast(dst, row0, nrows):
        """Broadcast gb rows [row0:row0+nrows] across partitions into dst."""
        src = bass.AP(tensor=dram["gb"], offset=row0 * D,
                      ap=[[0, P], [D, nrows], [1, D]])
        nc.gpsimd.dma_start(dst, src)

    with tile.TileContext(nc) as tc:
        with (
            tc.tile_pool(name="persist", bufs=1) as persist,
            tc.tile_pool(name="work", bufs=6) as work,
            tc.tile_pool(name="bias_sb", bufs=2) as bias_sb,
            tc.tile_pool(name="tp_ps", bufs=2, space="PSUM") as tp_ps,
            tc.tile_pool(name="dram_sc", bufs=1, space="DRAM") as dram_sc,
        ):
            ident = persist.tile([P, P], f32)
            nc.sync.dma_start(ident, dram["ident"].ap())
            rblob = persist.tile([1, res_total], f32r)
            nc.sync.dma_start(rblob, dram["resblob"].ap().bitcast(f32r))
            eps_t = persist.tile([P, 1], f32)
            nc.vector.memset(eps_t, 1e-5)

            def rb(name):
                off, size = res_offs[name]
                return rblob[0:1, off:off + size]

            ones_row = rb("ones")  # [1, 128] of 1.0

            def stream_bias(name):
                """DMA a one-shot bias vector into a rotating [1, *] tile."""
                off, size = str_offs[name]
                t = bias_sb.tile([1, HFF], f32r, name="bstr", tag="bstr")
                nc.sync.dma_start(
                    t[:, :size],
                    dram["strblob"].ap().bitcast(f32r)[0:1, off:off + size])
                return t[0:1, 0:size]

            def mm_group(ps, pairs):
                """Emit one PSUM accumulation group from (lhsT, rhs) pairs."""
                for i, (l, r) in enumerate(pairs):
                    nc.tensor.matmul(ps, l, r, start=(i == 0),
                                     stop=(i == len(pairs) - 1))

            def transpose_into(dstT, src_sb, n_chunks, tag):
                """src_sb [128, n*128] fp32 -> dstT [128, n, 128] fp32r."""
                for c in range(n_chunks):
                    pt = tp_ps.tile([P, P], f32, name="tp", tag="tp")
                    nc.tensor.transpose(pt, src_sb[:, c * P:(c + 1) * P], ident)
                    nc.vector.tensor_copy(dstT[:, c], pt)

            def layernorm(x_sb, out_sb, gbt, gi, bi):
                """out = LN(x) * gbt[:, gi] + gbt[:, bi]."""
                st = work.tile([P, 6], f32, name="lnst", tag="lnst")
                nc.vector.bn_stats(st, x_sb)
                mv = work.tile([P, 2], f32, name="lnmv", tag="lnmv")
                nc.vector.bn_aggr(mv, st)
                rstd = work.tile([P, 1], f32, name="lnrs", tag="lnrs")
                nc.scalar.activation(rstd, mv[:, 1:2], AF.Sqrt, bias=eps_t)
                nc.vector.reciprocal(rstd, rstd)
                nc.vector.tensor_scalar(out_sb, x_sb, mv[:, 0:1], rstd,
                                        op0=ALU.subtract, op1=ALU.mult)
                nc.vector.tensor_mul(out_sb, out_sb, gbt[:, gi])
                nc.vector.tensor_add(out_sb, out_sb, gbt[:, bi])

            # persistent activations
            emb = persist.tile([P, D], f32)
            embT = persist.tile([P, D // P, P], f32r)
            cw = persist.tile([P, E], f32)
            mixed = persist.tile([P, D], f32)
            h2T_dram = dram_sc.tile([S, P, 4, P], bf16)

            # ---------------- Encoder + gating ----------------
            with (
                tc.tile_pool(name="encw", bufs=1) as encw,
                tc.tile_pool(name="encs", bufs=2) as encs,
                tc.tile_pool(name="enc_ps", bufs=2, space="PSUM") as enc_ps,
                tc.tile_pool(name="enc_sb", bufs=1) as enc_sb,
            ):
                gbe = encw.tile([P, 6, D], f32)
                gb_bcast(gbe, 0, 6)
                w_proj = encw.tile([P, D], f32r)
                nc.sync.dma_start(w_proj, dram["proj_w"].ap().bitcast(f32r))
                w_attn = encw.tile([P, 4, D], f32r)
                nc.sync.dma_start(w_attn, kmaj("attn_w"))
                w_g1 = encw.tile([P, 4, G], f32r)
                nc.sync.dma_start(w_g1, kmaj("gate_w1"))
                w_g2 = encw.tile([P, 2, E], f32r)
                nc.sync.dma_start(w_g2, kmaj("gate_w2"))

                cons = enc_sb.tile([P, P], f32)
                nc.vector.memset(cons, 0.0)
                nc.sync.dma_start(cons[:, 0:F_IN], dram["constraints"].ap())
                consT = enc_sb.tile([P, P], f32r)
                pt0 = tp_ps.tile([P, P], f32, name="tp", tag="tp")
                nc.tensor.transpose(pt0, cons, ident)
                nc.vector.tensor_copy(consT, pt0)

                # h = LN0(cons @ proj_w + proj_b)
                ps = enc_ps.tile([P, D], f32, name="eps0", tag="emm")
                mm_group(ps, [(consT, w_proj),
                              (ones_row, stream_bias("proj_b"))])
                h_pre = enc_sb.tile([P, D], f32)
                nc.scalar.activation(h_pre, ps, AF.Copy)
                h_ln = enc_sb.tile([P, D], f32)
                layernorm(h_pre, h_ln, gbe, 0, 1)

                # attn (folded): x1 = h_ln + h_ln @ attn_w + b_attn; h1e = LN1
                hT = enc_sb.tile([P, 4, P], f32r)
                transpose_into(hT, h_ln, 4, "hT")
                ps = enc_ps.tile([P, D], f32, name="eps1", tag="emm")
                mm_group(ps, [(hT[:, k], w_attn[:, k]) for k in range(4)]
                         + [(ones_row, stream_bias("b_attn"))])
                x1 = enc_sb.tile([P, D], f32)
                nc.vector.tensor_tensor(x1, h_ln, ps, ALU.add)
                h1e = enc_sb.tile([P, D], f32)
                layernorm(x1, h1e, gbe, 2, 3)

                # ff: emb = LN2(h1e + gelu(h1e@W1+b1)@W2+b2), W1/W2 streamed
                h1eT = enc_sb.tile([P, 4, P], f32r)
                transpose_into(h1eT, h1e, 4, "h1eT")
                ffh = enc_sb.tile([P, HFF], f32)
                b1 = stream_bias("ff_b1")
                for nb in range(4):
                    wc = encs.tile([P, 4, D], f32r, name="ffw", tag="ffw")
                    nc.sync.dma_start(wc, kmaj("ff_w1")[:, :, nb * D:(nb + 1) * D])
                    psf = enc_ps.tile([P, D], f32, name="epsf", tag="emm")
                    mm_group(psf, [(h1eT[:, k], wc[:, k]) for k in range(4)]
                             + [(ones_row, b1[:, nb * D:(nb + 1) * D])])
                    nc.scalar.activation(ffh[:, nb * D:(nb + 1) * D], psf, AF.Gelu)
                ffhT = enc_sb.tile([P, 16, P], f32r)
                transpose_into(ffhT, ffh, 16, "ffhT")
                ps = enc_ps.tile([P, D], f32, name="eps2", tag="emm")
                for kb in range(4):
                    wc = encs.tile([P, 4, D], f32r, name="ffw2", tag="ffw")
                    nc.sync.dma_start(wc, kmaj("ff_w2")[:, kb * 4:(kb + 1) * 4, :])
                    for k in range(4):
                        nc.tensor.matmul(ps, ffhT[:, kb * 4 + k], wc[:, k],
                                         start=(kb == 0 and k == 0), stop=False)
                nc.tensor.matmul(ps, ones_row, stream_bias("ff_b2"),
                                 start=False, stop=True)
                x2 = enc_sb.tile([P, D], f32)
                nc.vector.tensor_tensor(x2, h1e, ps, ALU.add)
                layernorm(x2, emb, gbe, 4, 5)
                transpose_into(embT, emb, 4, "embT")

                # ---------------- Gating ----------------
                psg = enc_ps.tile([P, G], f32, name="epsg", tag="emm")
                mm_group(psg, [(embT[:, k], w_g1[:, k]) for k in range(4)]
                         + [(ones_row, rb("gate_b1"))])
                g1 = enc_sb.tile([P, G], f32)
                nc.scalar.activation(g1, psg, AF.Gelu)
                g1T = enc_sb.tile([P, 2, P], f32r)
                transpose_into(g1T, g1, 2, "g1T")
                psl = enc_ps.tile([P, E], f32, name="epsl", tag="emm")
                mm_group(psl, [(g1T[:, k], w_g2[:, k]) for k in range(2)]
                         + [(ones_row, rb("gate_b2"))])

                # softmax over E=8 (free axis)
                mx = work.tile([P, 1], f32, name="smx", tag="smx")
                nc.vector.reduce_max(mx, psl, axis=AX.X)
                sh = enc_sb.tile([P, E], f32)
                nc.vector.tensor_scalar(sh, psl, mx, None, op0=ALU.subtract)
                ex = enc_sb.tile([P, E], f32)
                nc.scalar.activation(ex, sh, AF.Exp)
                sm = work.tile([P, 1], f32, name="ssm", tag="ssm")
                nc.vector.reduce_sum(sm, ex, axis=AX.X)
                rs = work.tile([P, 1], f32, name="srs", tag="srs")
                nc.vector.reciprocal(rs, sm)
                probs = enc_sb.tile([P, E], f32)
                nc.vector.tensor_scalar_mul(probs, ex, rs)
                nc.sync.dma_start(probs_d.ap(), probs)

                # usage partial: column sums of probs via ones-column matmul
                ones_col = enc_sb.tile([P, 1], f32r)
                nc.sync.dma_start(ones_col, dram["ones_col"].ap().bitcast(f32r))
                probs_r = enc_sb.tile([P, E], f32r)
                nc.vector.tensor_copy(probs_r, probs)
                psu = enc_ps.tile([1, E], f32, name="epsu", tag="emm")
                nc.tensor.matmul(psu, ones_col, probs_r, start=True, stop=True)
                usage_sb = enc_sb.tile([1, E], f32)
                nc.vector.tensor_copy(usage_sb, psu)
                nc.sync.dma_start(usage_d.ap(), usage_sb)

                # top-2 combine weights cw[b, e] = p_e * in_top2 / (p1 + p2)
                m1 = work.tile([P, 1], f32, name="m1", tag="m1")
                nc.vector.reduce_max(m1, probs, axis=AX.X)
                mask1 = enc_sb.tile([P, E], f32)
                nc.vector.tensor_scalar(mask1, probs, m1, None, op0=ALU.is_equal)
                inv1 = enc_sb.tile([P, E], f32)
                nc.vector.tensor_scalar(inv1, mask1, -1.0, 1.0,
                                        op0=ALU.mult, op1=ALU.add)
                probs2 = enc_sb.tile([P, E], f32)
                nc.vector.tensor_mul(probs2, probs, inv1)
                m2 = work.tile([P, 1], f32, name="m2", tag="m2")
                nc.vector.reduce_max(m2, probs2, axis=AX.X)
                mask2 = enc_sb.tile([P, E], f32)
                nc.vector.tensor_scalar(mask2, probs2, m2, None, op0=ALU.is_equal)
                wmask = enc_sb.tile([P, E], f32)
                nc.vector.tensor_add(wmask, mask1, mask2)
                den = work.tile([P, 1], f32, name="den", tag="den")
                nc.vector.tensor_tensor(den, m1, m2, ALU.add)
                rden = work.tile([P, 1], f32, name="rden", tag="rden")
                nc.vector.reciprocal(rden, den)
                cwt = enc_sb.tile([P, E], f32)
                nc.vector.tensor_mul(cwt, probs, wmask)
                nc.vector.tensor_scalar_mul(cw, cwt, rden)

            # ------- Experts (bf16, interleaved between GRU steps) + GRU
            with (
                tc.tile_pool(name="expw", bufs=3) as expw,
                tc.tile_pool(name="exp_sb", bufs=2) as exp_sb,
                tc.tile_pool(name="exp_ps", bufs=2, space="PSUM") as exp_ps,
                tc.tile_pool(name="gruw", bufs=1) as gruw,
                tc.tile_pool(name="gru_sb", bufs=2) as gru_sb,
                tc.tile_pool(name="gru_gt", bufs=6) as gru_gt,
                tc.tile_pool(name="gru_ps", bufs=4, space="PSUM") as gru_ps,
            ):
                embT_bf = gruw.tile([P, 4, P], bf16)
                nc.vector.tensor_copy(embT_bf, embT)

                def expert_l1(e):
                    h1x = exp_sb.tile([P, HFF], f32, name="h1x", tag="h1x")
                    h1xT = exp_sb.tile([P, 16, P], bf16, name="h1xT",
                                       tag="h1xT")
                    b1e = stream_bias(f"exp_b1_{e}")
                    for nb in range(4):
                        w1c = expw.tile([P, 4, D], bf16, name="w1c", tag="ew")
                        nc.sync.dma_start(
                            w1c,
                            dram["exp_w1"].ap()[e, :, nb * D:(nb + 1) * D]
                            .rearrange("(o p) n -> p o n", p=P))
                        pse = exp_ps.tile([P, D], f32, name="pse", tag="eps")
                        mm_group(pse,
                                 [(embT_bf[:, k], w1c[:, k]) for k in range(4)]
                                 + [(ones_row, b1e[:, nb * D:(nb + 1) * D])])
                        nc.scalar.activation(h1x[:, nb * D:(nb + 1) * D],
                                             pse, AF.Gelu)
                        for c in range(4):
                            cc = nb * 4 + c
                            pt = tp_ps.tile([P, P], f32, name="tp", tag="tp")
                            nc.tensor.transpose(
                                pt, h1x[:, cc * P:(cc + 1) * P], ident)
                            nc.vector.tensor_copy(h1xT[:, cc], pt)
                    return h1xT

                def expert_l2(e, h1xT):
                    pso = exp_ps.tile([P, D], f32, name="pso", tag="eps")
                    for kb in range(4):
                        w2c = expw.tile([P, 4, D], bf16, name="w2c", tag="ew")
                        nc.sync.dma_start(
                            w2c,
                            dram["exp_w2"].ap()[e, kb * D:(kb + 1) * D, :]
                            .rearrange("(o p) n -> p o n", p=P))
                        for k in range(4):
                            nc.tensor.matmul(pso, h1xT[:, kb * 4 + k], w2c[:, k],
                                             start=(kb == 0 and k == 0),
                                             stop=False)
                    nc.tensor.matmul(pso, ones_row, stream_bias(f"exp_b2_{e}"),
                                     start=False, stop=True)
                    if e == 0:
                        nc.vector.tensor_scalar(mixed, pso, cw[:, 0:1], None,
                                                op0=ALU.mult)
                    else:
                        nc.vector.scalar_tensor_tensor(
                            mixed, pso, cw[:, e:e + 1], mixed,
                            op0=ALU.mult, op1=ALU.add)

                # ---- GRU weights + per-gate broadcast biases
                xseqT = gruw.tile([8, S, P], bf16)
                nc.sync.dma_start(xseqT, dram["xseq_aug"].ap())
                wi0a = gruw.tile([8, 3 * D], bf16)
                nc.sync.dma_start(wi0a, dram["wi0_aug"].ap())
                wh0 = gruw.tile([P, 4, 3 * D], bf16)
                nc.sync.dma_start(wh0, kmaj_bf("wh0"))
                wi1 = gruw.tile([P, 4, 3 * D], bf16)
                nc.sync.dma_start(wi1, kmaj_bf("wi1"))
                wh1 = gruw.tile([P, 4, 3 * D], bf16)
                nc.sync.dma_start(wh1, kmaj_bf("wh1"))
                # grub rows: 0 c2-r bias, 1 c2-z bias, 2 c2-inn, 3 c1-hn, 4 c2-hn
                grub = gruw.tile([P, 5, D], f32)
                nc.gpsimd.dma_start(
                    grub, bass.AP(tensor=dram["grub"], offset=0,
                                  ap=[[0, P], [D, 5], [1, D]]))

                def gru_cell(ci, gates_mms, h_sb, h1p_sb, rz_bias, i_bias,
                             h_bias):
                    """sigma-only GRU cell (tanh(v) = 2*sigmoid(2v) - 1 keeps
                    the sigmoid LUT resident on ACT). h1p_sb = h_sb + 1.
                    Returns (h_new, h_new + 1)."""
                    pss = {}
                    for gname in ("r", "z", "i", "h"):
                        ps = gru_ps.tile([P, D], f32, name=f"ps{gname}{ci}",
                                         tag="gps")
                        mm_group(ps, gates_mms[gname])
                        pss[gname] = ps
                    if rz_bias is not None:
                        rpre = gru_gt.tile([P, D], f32, name=f"rp{ci}", tag="gt")
                        nc.vector.tensor_tensor(rpre, pss["r"], rz_bias[0],
                                                ALU.add)
                        zpre = gru_gt.tile([P, D], f32, name=f"zp{ci}", tag="gt")
                        nc.vector.tensor_tensor(zpre, pss["z"], rz_bias[1],
                                                ALU.add)
                    else:
                        rpre, zpre = pss["r"], pss["z"]
                    r_sb = gru_gt.tile([P, D], f32, name=f"r{ci}", tag="gt")
                    nc.scalar.activation(r_sb, rpre, AF.Sigmoid)
                    z_sb = gru_gt.tile([P, D], f32, name=f"z{ci}", tag="gt")
                    nc.scalar.activation(z_sb, zpre, AF.Sigmoid)
                    hb = gru_gt.tile([P, D], f32, name=f"hb{ci}", tag="gt")
                    nc.vector.tensor_tensor(hb, pss["h"], h_bias, ALU.add)
                    t1 = gru_gt.tile([P, D], f32, name=f"t1{ci}", tag="gt")
                    nc.vector.tensor_mul(t1, r_sb, hb)
                    nc.vector.tensor_tensor(t1, t1, pss["i"], ALU.add)
                    if i_bias is not None:
                        nc.vector.tensor_tensor(t1, t1, i_bias, ALU.add)
                    # n = tanh(t1) = 2*s - 1 with s = sigmoid(2*t1)
                    s_sb = gru_gt.tile([P, D], f32, name=f"s{ci}", tag="gt")
                    nc.scalar.activation(s_sb, t1, AF.Sigmoid, scale=2.0)
                    # h_new = n + z*(h - n); h - n = (h + 1) - 2s
                    dd = gru_gt.tile([P, D], f32, name=f"d{ci}", tag="gt")
                    nc.vector.scalar_tensor_tensor(dd, s_sb, -2.0, h1p_sb,
                                                   op0=ALU.mult, op1=ALU.add)
                    nc.vector.tensor_mul(dd, z_sb, dd)
                    hnew = gru_sb.tile([P, D], f32, name=f"hs{ci}",
                                       tag=f"hs{ci}")
                    nc.vector.scalar_tensor_tensor(hnew, s_sb, 2.0, dd,
                                                   op0=ALU.mult, op1=ALU.add)
                    nc.vector.tensor_scalar_add(hnew, hnew, -1.0)
                    h1p_new = gru_sb.tile([P, D], f32, name=f"hp{ci}",
                                          tag=f"hp{ci}")
                    nc.vector.tensor_scalar_add(h1p_new, hnew, 1.0)
                    return hnew, h1p_new

                h1_sb = gru_sb.tile([P, D], f32, name="hs1", tag="hs1")
                nc.vector.memset(h1_sb, 0.0)
                h2_sb = gru_sb.tile([P, D], f32, name="hs2", tag="hs2")
                nc.vector.memset(h2_sb, 0.0)
                h1p_sb = gru_sb.tile([P, D], f32, name="hp1", tag="hp1")
                nc.vector.memset(h1p_sb, 1.0)
                h2p_sb = gru_sb.tile([P, D], f32, name="hp2", tag="hp2")
                nc.vector.memset(h2p_sb, 1.0)
                h1T = None
                h2T_prev = None

                for t in range(S):
                    # interleave half an expert per step so its matmuls fill
                    # PE gaps in the recurrence and its weight DMA streams
                    # behind the compute
                    if t // 2 < E:
                        if t % 2 == 0:
                            cur_h1xT = expert_l1(t // 2)
                        else:
                            expert_l2(t // 2, cur_h1xT)

                    xs = xseqT[:, t, :]
                    g1m = {
                        "r": [(xs, wi0a[:, 0:D])],
                        "z": [(xs, wi0a[:, D:2 * D])],
                        "i": [(xs, wi0a[:, 2 * D:3 * D])],
                        "h": [],
                    }
                    if h1T is not None:
                        for k in range(4):
                            g1m["r"].append((h1T[:, k], wh0[:, k, 0:D]))
                            g1m["z"].append((h1T[:, k], wh0[:, k, D:2 * D]))
                            g1m["h"].append((h1T[:, k], wh0[:, k, 2 * D:3 * D]))
                    else:
                        g1m["h"] = [(ones_row, rb("zeroD"))]
                    h1_sb, h1p_sb = gru_cell(1, g1m, h1_sb, h1p_sb, None, None, grub[:, 3])
                    h1T = gru_sb.tile([P, 4, P], bf16, name="h1T", tag="h1T")
                    transpose_into(h1T, h1_sb, 4, "h1T")

                    g2m = {"r": [], "z": [], "i": [], "h": []}
                    if h2T_prev is not None:
                        for k in range(4):
                            g2m["r"].append((h2T_prev[:, k], wh1[:, k, 0:D]))
                            g2m["z"].append((h2T_prev[:, k],
                                             wh1[:, k, D:2 * D]))
                            g2m["h"].append((h2T_prev[:, k],
                                             wh1[:, k, 2 * D:3 * D]))
                    else:
                        g2m["h"] = [(ones_row, rb("zeroD"))]
                    for k in range(4):
                        g2m["r"].append((h1T[:, k], wi1[:, k, 0:D]))
                        g2m["z"].append((h1T[:, k], wi1[:, k, D:2 * D]))
                        g2m["i"].append((h1T[:, k], wi1[:, k, 2 * D:3 * D]))
                    h2_sb, h2p_sb = gru_cell(2, g2m, h2_sb, h2p_sb,
                                             (grub[:, 0], grub[:, 1]),
                                             grub[:, 2], grub[:, 4])
                    h2T = gru_sb.tile([P, 4, P], bf16, name="h2T", tag="h2T")
                    transpose_into(h2T, h2_sb, 4, "h2T")
                    nc.sync.dma_start(h2T_dram[t], h2T)
                    h2T_prev = h2T

            # ---------------- context = gelu(LN(mixed@mix_w + mix_b)) + emb
            ctx = persist.tile([P, D], f32)
            ctxT = persist.tile([P, D // P, P], bf16)
            with (
                tc.tile_pool(name="mixw", bufs=1) as mixw,
                tc.tile_pool(name="mix_ps", bufs=2, space="PSUM") as mix_ps,
                tc.tile_pool(name="mix_sb", bufs=1) as mix_sb,
            ):
                gbm = mixw.tile([P, 2, D], f32)
                gb_bcast(gbm, 6, 2)
                w_mix = mixw.tile([P, 4, D], f32r)
                nc.sync.dma_start(w_mix, kmaj("mix_w"))
                mixT = mix_sb.tile([P, 4, P], f32r)
                transpose_into(mixT, mixed, 4, "mixT")
                ps = mix_ps.tile([P, D], f32, name="mps", tag="mmm")
                mm_group(ps, [(mixT[:, k], w_mix[:, k]) for k in range(4)]
                         + [(ones_row, stream_bias("mix_b"))])
                mx_sb = mix_sb.tile([P, D], f32)
                nc.scalar.activation(mx_sb, ps, AF.Copy)
                mxn = mix_sb.tile([P, D], f32)
                layernorm(mx_sb, mxn, gbm, 0, 1)
                mxg = mix_sb.tile([P, D], f32)
                nc.scalar.activation(mxg, mxn, AF.Gelu)
                nc.vector.tensor_tensor(ctx, mxg, emb, ALU.add)
                transpose_into(ctxT, ctx, 4, "ctxT")

            # ---------------- decoder heads, staged so each ACT function
            # runs as one homogeneous batch (one LUT load per function)
            with (
                tc.tile_pool(name="combw", bufs=1) as combw,
                tc.tile_pool(name="cmb_ps", bufs=4, space="PSUM") as cmb_ps,
                tc.tile_pool(name="cmb_sb", bufs=3) as cmb_sb,
            ):
                gbc = combw.tile([P, 2, D], f32)
                gb_bcast(gbc, 8, 2)
                w_comb = combw.tile([P, 8, D], bf16)
                nc.sync.dma_start(w_comb, kmaj_bf("comb_w"))
                w_head = combw.tile([P, 4, OUT_F + 1], bf16)
                nc.sync.dma_start(w_head, kmaj_bf("head_w"))
                cm_all = combw.tile([P, S, D], f32)
                rooms_raw = combw.tile([P, S, OUT_F + 1], f32)

                h2Tts = []
                for t in range(S):
                    h2Tt = cmb_sb.tile([P, 4, P], bf16, name="h2Tt",
                                       tag=f"h2Tt{t % 4}")
                    nc.sync.dma_start(h2Tt, h2T_dram[t])
                    h2Tts.append(h2Tt)
                    ps = cmb_ps.tile([P, D], f32, name="cps", tag="cmm")
                    mm_group(ps,
                             [(ctxT[:, k], w_comb[:, k]) for k in range(4)]
                             + [(h2Tt[:, k], w_comb[:, 4 + k])
                                for k in range(4)]
                             + [(ones_row, rb("comb_b"))])
                    nc.vector.tensor_copy(cm_all[:, t], ps)
                for t in range(S):
                    layernorm(cm_all[:, t], cm_all[:, t], gbc, 0, 1)
                for t in range(S):
                    nc.scalar.activation(cm_all[:, t], cm_all[:, t], AF.Gelu)
                for t in range(S):
                    hidT = cmb_sb.tile([P, 4, P], bf16, name="hidT",
                                       tag="hidT")
                    transpose_into(hidT, cm_all[:, t], 4, "hidT")
                    psh = cmb_ps.tile([P, OUT_F + 1], f32, name="hps",
                                      tag="cmm")
                    mm_group(psh,
                             [(hidT[:, k], w_head[:, k]) for k in range(4)]
                             + [(ones_row, rb("head_b"))])
                    nc.vector.tensor_copy(rooms_raw[:, t], psh)
                nc.scalar.activation(rooms_raw[:, :, 0:4], rooms_raw[:, :, 0:4],
                                     AF.Sigmoid)
                nc.scalar.activation(rooms_raw[:, :, 20:22],
                                     rooms_raw[:, :, 20:22], AF.Sigmoid)
                nc.sync.dma_start(rooms_d.ap(), rooms_raw[:, :, 0:OUT_F])

    _legalize_waits(nc)
    return nc


_BUILD_CACHE = {}
LAST_EXEC_NS = None


def _make_blobs(p):
    res = _Blob()
    res.add("ones", np.ones(P))
    res.add("zeroD", np.zeros(D))
    res.add("gate_b1", p["gate_b1"])
    res.add("gate_b2", p["gate_b2"])
    res.add("bh0_n", p["gru_bh0"][2 * D:])
    res.add("bi1bh1_rz", (p["gru_bi1"] + p["gru_bh1"])[:2 * D])
    res.add("bi1_n", p["gru_bi1"][2 * D:])
    res.add("bh1_n", p["gru_bh1"][2 * D:])
    res.add("comb_b", p["comb_b"])
    res.add("head_b", np.concatenate(
        [p["coord_b"], p["type_b"], p["zone_b"], p["ext_b"], p["stop_b"],
         np.zeros(1, np.float32)]))

    st = _Blob()
    st.add("proj_b", p["proj_b"])
    st.add("b_attn", p["attn_bv"] @ p["attn_wo"] + p["attn_bo"])
    st.add("ff_b1", p["ff_b1"])
    st.add("ff_b2", p["ff_b2"])
    st.add("mix_b", p["mix_b"])
    for e in range(E):
        st.add(f"exp_b1_{e}", p["exp_b1"][e])
        st.add(f"exp_b2_{e}", p["exp_b2"][e])
    return res, st


def kernel(constraints, teacher_rooms, params, num_rooms):
    constraints = np.ascontiguousarray(np.asarray(constraints, np.float32))
    teacher_rooms = np.ascontiguousarray(np.asarray(teacher_rooms, np.float32))
    p = {k: np.ascontiguousarray(np.asarray(v, np.float32))
         for k, v in params.items()}
    S = int(num_rooms)
    B = constraints.shape[0]
    Bl = B // N_CORES
    import ml_dtypes
    bfl = ml_dtypes.bfloat16

    res, st = _make_blobs(p)

    gb = np.ascontiguousarray(np.stack(
        [p["ln0_g"], p["ln0_b"], p["ln1_g"], p["ln1_b"],
         p["ln2_g"], p["ln2_b"], p["mix_ln_g"], p["mix_ln_b"],
         p["comb_ln_g"], p["comb_ln_b"]]))

    proj_w_pad = np.zeros((P, D), np.float32)
    proj_w_pad[:F_IN] = p["proj_w"]
    attn_w = np.ascontiguousarray(p["attn_wv"] @ p["attn_wo"])

    wi0_aug = np.zeros((8, 3 * D), np.float32)
    wi0_aug[:RF] = p["gru_wi0"]
    wi0_aug[RF, :2 * D] = (p["gru_bi0"] + p["gru_bh0"])[:2 * D]
    wi0_aug[RF, 2 * D:] = p["gru_bi0"][2 * D:]

    head_w = np.ascontiguousarray(np.concatenate(
        [p["coord_w"], p["type_w"], p["zone_w"], p["ext_w"], p["stop_w"],
         np.zeros((D, 1), np.float32)], 1))

    rzsum = (p["gru_bi1"] + p["gru_bh1"])
    grub = np.ascontiguousarray(np.stack(
        [rzsum[0:D], rzsum[D:2 * D], p["gru_bi1"][2 * D:],
         p["gru_bh0"][2 * D:], p["gru_bh1"][2 * D:]]))

    key = S
    if key not in _BUILD_CACHE:
        _BUILD_CACHE[key] = _build(S, res.offs, res.total, st.offs, st.total)
    nc = _BUILD_CACHE[key]

    shared = {
        "ident": np.eye(P, dtype=np.float32),
        "ones_col": np.ones((P, 1), np.float32),
        "resblob": np.ascontiguousarray(res.array()),
        "strblob": np.ascontiguousarray(st.array()),
        "gb": gb,
        "proj_w": proj_w_pad,
        "attn_w": attn_w,
        "ff_w1": p["ff_w1"], "ff_w2": p["ff_w2"],
        "gate_w1": p["gate_w1"], "gate_w2": p["gate_w2"],
        "exp_w1": np.ascontiguousarray(p["exp_w1"].astype(bfl)),
        "exp_w2": np.ascontiguousarray(p["exp_w2"].astype(bfl)),
        "grub": grub,
        "mix_w": p["mix_w"],
        "wi0_aug": np.ascontiguousarray(wi0_aug.astype(bfl)),
        "wh0": np.ascontiguousarray(p["gru_wh0"].astype(bfl)),
        "wi1": np.ascontiguousarray(p["gru_wi1"].astype(bfl)),
        "wh1": np.ascontiguousarray(p["gru_wh1"].astype(bfl)),
        "comb_w": np.ascontiguousarray(p["comb_w"].astype(bfl)),
        "head_w": np.ascontiguousarray(head_w.astype(bfl)),
    }

    in_maps = []
    for i in range(N_CORES):
        sl = slice(i * Bl, (i + 1) * Bl)
        xseq_aug = np.zeros((8, S, P), np.float32)
        tr = teacher_rooms[sl]
        for t in range(1, S):
            xseq_aug[:RF, t, :] = tr[:, t - 1, :].T
        xseq_aug[RF, :, :] = 1.0
        m = dict(shared)
        m["constraints"] = np.ascontiguousarray(constraints[sl])
        m["xseq_aug"] = xseq_aug.astype(bfl)
        in_maps.append(m)

    global LAST_EXEC_NS
    profile = bool(os.environ.get("KERNEL_PROFILE"))
    res_hw = run_bass_kernel_spmd(nc, in_maps, core_ids=list(range(N_CORES)),
                                  trace=profile)
    LAST_EXEC_NS = res_hw.exec_time_ns

    rooms = np.concatenate([r["rooms"] for r in res_hw.results], 0)
    gate_probs = np.concatenate([r["gate_probs"] for r in res_hw.results], 0)
    usage = sum(r["usage_p"][0] for r in res_hw.results) / B
    aux = np.float32(np.mean((usage - 1.0 / E) ** 2) * E)
    return rooms, gate_probs, aux
